# revision 19
# baseline (speedup 1.0000x reference)
"""GAT message-passing GNN on 8 Trainium2 NeuronCores (Bass/Tile).

Strategy: nodes are permuted (degree-balanced, round-robin over 160 tiles of
125 nodes) and partitioned across 8 cores (20 dst tiles each). Each layer:
every core redundantly computes xl = h @ W (and attention logits al = h @ WA)
for all nodes into a DRAM gather table; each core then processes its own dst
tiles: one dma_gather fetches xl[src] (+al_src) for all incident edges,
per-128-edge-block one-hot dst matrices are built with is_equal compares, and
the segment softmax + weighted scatter-add run as f32r matmuls accumulating in
PSUM (w = exp(leakyrelu(as+ad)) per edge; out = S^T @ (w*X); z = S^T @ w;
divide by z once per dst node). Updated node features are AllGathered each
layer. Final graph mean-pool is a one-hot matmul + AllReduce, then the MLP.
"""
import numpy as np

import concourse.bass as bass
import concourse.bacc as bacc
import concourse.mybir as mybir
import concourse.tile as tile
from concourse.bass_utils import run_bass_kernel_spmd

F32 = mybir.dt.float32
F32R = mybir.dt.float32r
BF16 = mybir.dt.bfloat16
I16 = mybir.dt.int16
AF = mybir.ActivationFunctionType
ALU = mybir.AluOpType

N, E, FIN, HID, HEADS, L, G = 20000, 200000, 20, 128, 4, 4, 32
NEG = 0.2
NCORE = 8
NT = 160            # global dst tiles
TPC = NT // NCORE   # 20 tiles per core
TILE_N = N // NT    # 125 real nodes per tile
PN = NT * 128       # padded node id space; PN = zero row
DROW = 640          # bf16 table row: 512 xl + 8 (4 f32 al_src) + pad (1280B, %256==0)
PADDST = 999.0
TPC_DUMP = 20  # dump tiles 0,20,40,... (first tile of each core)

_ZERO_WAIT_OPCODES = (
    "InstDMAGatherAnt",
    "InstDMAScatterAddAnt",
    "InstPartitionBroadcast",
    "InstPartitionAllReduce",
    "InstAPGather",
    "InstIndirectCopy",
    "InstSparseGather",
    "InstGatherTranspose",
)
_spill_counter = [0]


def _split_waits(nc, default_limit=1):
    """Spill excess semaphore waits onto preceding same-engine EventSemaphore
    instructions (walrus wait-slot limits: 0 for extended DMA ops, ~1+ else)."""
    for f in nc.m.functions:
        for bb in f.blocks:
            out = []
            changed = False
            for ins in bb.instructions:
                si = ins.sync_info
                waits = list(si.on_wait) if si is not None and si.on_wait else []
                tname = type(ins).__name__
                limit = default_limit
                if tname in _ZERO_WAIT_OPCODES:
                    limit = 0
                elif ins.engine == mybir.EngineType.Pool and tname in (
                    "InstDrain",
                    "InstNoOp",
                ):
                    limit = 0
                if len(waits) > limit:
                    changed = True
                    keep = waits[:limit] if limit else []
                    spill = waits[limit:] if limit else waits
                    while spill:
                        chunk, spill = spill[:1], spill[1:]
                        _spill_counter[0] += 1
                        nop = mybir.InstEventSemaphore(
                            name=f"waitspill-{_spill_counter[0]}"
                        )
                        nop.engine = ins.engine
                        nop.sync_info = mybir.SyncInfo(on_wait=chunk, on_update=[])
                        nc.register_instruction(nop, overwrite=True)
                        out.append(nop)
                    ins.sync_info = mybir.SyncInfo(
                        on_wait=keep, on_update=list(si.on_update) if si else []
                    )
                out.append(ins)
            if changed:
                bb.instructions[:] = out


def _preprocess(x, edge_index, batch, gat_W, att_src, att_dst):
    """Degree-balanced node permutation + per-core edge/tile data."""
    src = np.concatenate([edge_index[0], np.arange(N, dtype=np.int64)])
    dst = np.concatenate([edge_index[1], np.arange(N, dtype=np.int64)])
    indeg = np.bincount(dst, minlength=N)
    order = np.argsort(-indeg, kind="stable")
    new_id = np.empty(N, dtype=np.int64)
    ranks = np.arange(N)
    new_id[order] = (ranks % NT) * 128 + (ranks // NT)

    nsrc = new_id[src]
    ndst = new_id[dst]
    tile_e = ndst >> 7
    dloc = ndst & 127
    eorder = np.argsort(tile_e, kind="stable")
    tile_sorted = tile_e[eorder]
    nsrc_sorted = nsrc[eorder]
    dloc_sorted = dloc[eorder]
    starts = np.searchsorted(tile_sorted, np.arange(NT + 1))
    cnts = np.diff(starts)
    nblk = int(np.ceil(cnts.max() / 128))
    ET = nblk * 128

    gsrc = np.full((NT, ET), PN, dtype=np.int64)
    gdst = np.full((NT, ET), int(PADDST), dtype=np.int64)
    for t in range(NT):
        s, c = starts[t], cnts[t]
        gsrc[t, :c] = nsrc_sorted[s : s + c]
        gdst[t, :c] = dloc_sorted[s : s + c]

    # per-core arrays
    gidx = np.zeros((NCORE, 128, TPC * nblk * 8), dtype=np.int16)
    dcols = np.zeros((NCORE, 128, TPC * nblk), dtype=np.float32)
    for c in range(NCORE):
        for tl in range(TPC):
            t = c * TPC + tl
            wrap = gsrc[t].astype(np.int16).reshape(ET // 16, 16).T  # [16, ET/16]
            gidx[c, :, tl * nblk * 8 : (tl + 1) * nblk * 8] = np.tile(wrap, (8, 1))
            dcols[c, :, tl * nblk : (tl + 1) * nblk] = (
                gdst[t].reshape(nblk, 128).T.astype(np.float32)
            )

    # pooling matrix with 1/cnt folded in
    cnt = np.bincount(batch, minlength=G).astype(np.float32)
    cnt = np.maximum(cnt, 1.0)
    btile = np.zeros((NCORE, 128, TPC * 32), dtype=np.float32)
    inv = np.zeros(PN, dtype=np.int64)
    inv[new_id] = np.arange(N)  # new -> old (only valid slots)
    valid = np.zeros(PN, dtype=bool)
    valid[new_id] = True
    for c in range(NCORE):
        for tl in range(TPC):
            t = c * TPC + tl
            for p in range(TILE_N):
                nid = t * 128 + p
                if valid[nid]:
                    n_old = inv[nid]
                    g = batch[n_old]
                    btile[c, p, tl * 32 + g] = 1.0 / cnt[g]

    # permuted transposed input features
    xT = np.zeros((FIN, PN), dtype=np.float32)
    xT[:, new_id] = x.T

    # folded attention projections WA_l = W_l @ [A_src | A_dst]
    WA = np.zeros((L, HID, 2 * HEADS), dtype=np.float32)
    for l in range(L):
        A = np.zeros((HID * HEADS, 2 * HEADS), dtype=np.float64)
        for h in range(HEADS):
            A[h * HID : (h + 1) * HID, h] = att_src[l][h]
            A[h * HID : (h + 1) * HID, HEADS + h] = att_dst[l][h]
        WA[l] = (gat_W[l].astype(np.float64) @ A).astype(np.float32)

    return dict(gidx=gidx, dcols=dcols, btile=btile, xT=xT, WA=WA, nblk=nblk)


def _build(nblk, dbg_stop=None, dbg_layers=L, dbg_pad=0):
    ET = nblk * 128
    nc = bacc.Bacc("TRN2", target_bir_lowering=False, debug=False, num_devices=NCORE)

    t_xT = nc.dram_tensor("xT", [FIN, PN], F32, kind="ExternalInput")
    t_gidx = nc.dram_tensor("gidx", [128, TPC * nblk * 8], I16, kind="ExternalInput")
    t_dcols = nc.dram_tensor("dcols", [128, TPC * nblk], F32, kind="ExternalInput")
    t_dcolsb = nc.dram_tensor("dcolsb", [128, TPC * nblk], BF16, kind="ExternalInput")
    t_iota_rowb = nc.dram_tensor("iota_rowb", [128, 128], BF16, kind="ExternalInput")
    t_btile = nc.dram_tensor("btile", [128, TPC * 32], F32R, kind="ExternalInput")
    t_iota_row = nc.dram_tensor("iota_row", [128, 128], F32, kind="ExternalInput")
    t_iota_col = nc.dram_tensor("iota_col", [128, 1], F32, kind="ExternalInput")
    t_ident = nc.dram_tensor("ident", [128, 128], F32, kind="ExternalInput")
    t_identr = nc.dram_tensor("identr", [128, 128], F32R, kind="ExternalInput")
    t_Win = nc.dram_tensor("Win", [FIN, HID], F32, kind="ExternalInput")
    t_Wl = nc.dram_tensor("Wl", [L, HID, HEADS * HID], F32R, kind="ExternalInput")
    t_WAl = nc.dram_tensor("WAl", [L, HID, 2 * HEADS], F32R, kind="ExternalInput")
    t_btl = nc.dram_tensor("btl", [L, 128, HID], F32, kind="ExternalInput")
    t_W1 = nc.dram_tensor("W1", [HID, 64], F32R, kind="ExternalInput")
    t_W2 = nc.dram_tensor("W2", [64, 64], F32R, kind="ExternalInput")
    t_W3 = nc.dram_tensor("W3", [64, 32], F32R, kind="ExternalInput")
    t_b1 = nc.dram_tensor("b1t", [32, 64], F32, kind="ExternalInput")
    t_b2 = nc.dram_tensor("b2t", [32, 64], F32, kind="ExternalInput")
    t_b3 = nc.dram_tensor("b3t", [32, 32], F32, kind="ExternalInput")
    o_out = nc.dram_tensor("out", [G, 32], F32, kind="ExternalOutput")
    o_h = nc.dram_tensor("hdump", [128, 8 * HID], F32, kind="ExternalOutput")

    table = nc.dram_tensor("table", [PN + 1, DROW], BF16)

    with tile.TileContext(nc) as tc:
        with (
            tc.tile_pool(name="const", bufs=1) as cpool,
            tc.tile_pool(name="persist", bufs=1) as hpool,
            tc.tile_pool(name="dram", bufs=1, space="DRAM") as dpool,
        ):
            # ---- constants to SBUF ----
            iota_row = cpool.tile([128, 128], F32)
            iota_col = cpool.tile([128, 1], F32)
            ident = cpool.tile([128, 128], F32)
            identr = cpool.tile([128, 128], F32R)
            Win = cpool.tile([FIN, HID], F32)
            Wl = cpool.tile([128, L, HEADS * HID], F32R)
            WAl = cpool.tile([128, L, 2 * HEADS], F32R)
            btl = cpool.tile([128, L, HID], F32)
            gidx = cpool.tile([128, TPC * nblk * 8], I16)
            dcols = cpool.tile([128, TPC * nblk], F32)
            dcolsb = cpool.tile([128, TPC * nblk], BF16)
            iota_rowb = cpool.tile([128, 128], BF16)
            btile = cpool.tile([128, TPC * 32], F32R)
            nc.sync.dma_start(iota_row[:], t_iota_row[:])
            nc.sync.dma_start(iota_col[:], t_iota_col[:])
            nc.sync.dma_start(ident[:], t_ident[:])
            nc.sync.dma_start(identr[:], t_identr[:])
            nc.sync.dma_start(Win[:], t_Win[:])
            for l in range(L):
                nc.sync.dma_start(Wl[:, l, :], t_Wl[l])
                nc.sync.dma_start(WAl[:, l, :], t_WAl[l])
                nc.sync.dma_start(btl[:, l, :], t_btl[l])
            nc.gpsimd.dma_start(gidx[:], t_gidx[:])
            nc.sync.dma_start(dcols[:], t_dcols[:])
            nc.sync.dma_start(dcolsb[:], t_dcolsb[:])
            nc.sync.dma_start(iota_rowb[:], t_iota_rowb[:])
            nc.sync.dma_start(btile[:], t_btile[:])

            # persistent node features [p, tile, c]
            h_res = hpool.tile([128, NT, HID], F32R)
            al_dst_all = hpool.tile([128, NT, HEADS], BF16)
            al_dst_own = hpool.tile([128, TPC, HEADS], BF16)

            # zero row of the gather table
            with tc.tile_pool(name="zr", bufs=1) as zpool:
                zrow = zpool.tile([1, DROW], BF16)
                nc.vector.memset(zrow[:], 0.0)
                nc.sync.dma_start(table[PN : PN + 1, :], zrow[:])

            # collective buffers
            cc_in = dpool.tile([TPC, 128, HID], F32R)
            ag_outs = [
                dpool.tile([NT, 128, HID], F32R, addr_space="Shared", name=f"ag{i}")
                for i in range(L)
            ]
            ar_in = dpool.tile([G, HID], F32)
            ar_out = dpool.tile([G, HID], F32, addr_space="Shared")

            pid = nc.vector.partition_id()

            if dbg_pad:
                with tc.tile_pool(name="padp", bufs=2) as padp:
                    pa = padp.tile([1, 16], F32, tag="pa")
                    nc.vector.memset(pa[:], 0.0)
                    for _ in range(dbg_pad):
                        pb = padp.tile([1, 16], F32, tag="pa")
                        nc.vector.tensor_copy(pb[:], pa[:])
                        pa = pb
            # ---- phase 0: h0 = relu(x @ Win) ----
            with (
                tc.tile_pool(name="p0s", bufs=3) as p0s,
                tc.tile_pool(name="p0p", bufs=2, space="PSUM") as p0p,
            ):
                for t in range(NT):
                    xt = p0s.tile([FIN, 128], F32, tag="xt")
                    nc.sync.dma_start(xt[:], t_xT[:, t * 128 : (t + 1) * 128])
                    ph = p0p.tile([128, HID], F32, tag="ph")
                    nc.tensor.matmul(ph[:], xt[:], Win[:], start=True, stop=True)
                    nc.scalar.activation(h_res[:, t, :], ph[:], AF.Relu)

            for l in range(dbg_layers):
                # ---- P1: xl/al for all nodes -> gather table ----
                with (
                    tc.tile_pool(name="p1s", bufs=3) as p1s,
                    tc.tile_pool(name="p1p", bufs=2, space="PSUM") as p1p,
                ):
                    for t in range(NT):
                        hT_ps = p1p.tile([128, 128], F32R, tag="hT")
                        nc.tensor.transpose(hT_ps[:], h_res[:, t, :], identr[:])
                        hT = p1s.tile([128, 128], F32R, tag="hT")
                        nc.vector.tensor_copy(hT[:], hT_ps[:])
                        pxl = p1p.tile([128, HEADS * HID], F32, tag="xl")
                        nc.tensor.matmul(
                            pxl[:], hT[:], Wl[:, l, :], start=True, stop=True
                        )
                        pal = p1p.tile([128, 2 * HEADS], F32, tag="al")
                        nc.tensor.matmul(
                            pal[:], hT[:], WAl[:, l, :], start=True, stop=True
                        )
                        stage = p1s.tile([128, DROW], BF16, tag="st")
                        if t % 3 == 0:
                            nc.vector.tensor_copy(stage[:, 0:512], pxl[:])
                        else:
                            nc.scalar.copy(stage[:, 0:512], pxl[:])
                        nc.vector.tensor_copy(
                            stage[:, 512:520].bitcast(F32), pal[:, 0:HEADS]
                        )
                        nc.vector.tensor_copy(
                            al_dst_all[:, t, :], pal[:, HEADS : 2 * HEADS]
                        )
                        nc.sync.dma_start(
                            table[t * 128 : (t + 1) * 128, 0:520], stage[:, 0:520]
                        )
                    # own slice of al_dst (core-dependent via register offset)
                    nc.vector.tensor_copy(
                        al_dst_own[:, :, :],
                        al_dst_all[:, bass.ds(pid * TPC, TPC), :],
                    )

                if dbg_stop == "p1":
                    break
                # ---- P2: per own dst tile: gather + attention + scatter ----
                with (
                    tc.tile_pool(name="p2s", bufs=2) as p2s,
                    tc.tile_pool(name="p2p", bufs=2, space="PSUM") as p2p,
                ):
                    for tl in range(TPC):
                        X = p2s.tile([128, nblk, DROW], BF16, tag="X")
                        for g0 in range(0, nblk, 8):
                            g1 = min(g0 + 8, nblk)
                            nc.gpsimd.dma_gather(
                                X[:, g0:g1, :],
                                table[:],
                                gidx[
                                    :,
                                    tl * nblk * 8 + g0 * 8 : tl * nblk * 8 + g1 * 8,
                                ],
                                (g1 - g0) * 128,
                                (g1 - g0) * 128,
                                DROW,
                            )
                        pad = p2p.tile([128, nblk * HEADS], F32, tag="ad")
                        pout = p2p.tile([128, HEADS * HID], F32, tag="out")
                        pz = p2p.tile([128, HEADS], F32, tag="z")
                        Sb_l = []
                        for b in range(nblk):
                            dcol = dcols[:, tl * nblk + b : tl * nblk + b + 1]
                            drows = p2p.tile([128, 128], F32, tag="dr")
                            nc.tensor.transpose(
                                drows[:], dcol.to_broadcast([128, 128]), ident[:]
                            )
                            Sb = p2s.tile([128, 128], BF16, tag=f"Sb{b % 2}")
                            SbT = p2s.tile([128, 128], BF16, tag=f"SbT{b % 2}")
                            nc.vector.tensor_scalar(
                                Sb[:], iota_rowb[:], dcol, None, ALU.is_equal
                            )
                            nc.vector.tensor_scalar(
                                SbT[:], drows[:], iota_col[:], None, ALU.is_equal
                            )
                            nc.tensor.matmul(
                                pad[:, b * HEADS : (b + 1) * HEADS],
                                SbT[:],
                                al_dst_own[:, tl, :],
                                start=True,
                                stop=True,
                            )
                            Sb_l.append(Sb)
                        # attention weights for the whole tile
                        ew = p2s.tile([128, nblk * HEADS], F32, tag="ew")
                        nc.vector.tensor_add(
                            ew[:],
                            X[:, :, 512:520].bitcast(F32),
                            pad[:],
                        )
                        nc.vector.scalar_tensor_tensor(
                            ew[:], ew[:], NEG, ew[:], ALU.mult, ALU.max
                        )
                        nc.scalar.activation(ew[:], ew[:], AF.Exp)
                        ewr = p2s.tile([128, nblk * HEADS], BF16, tag="ewr")
                        nc.vector.tensor_copy(ewr[:], ew[:])
                        for b in range(nblk):
                            wX = p2s.tile([128, HEADS * HID], BF16, tag=f"wX{b % 2}")
                            for hh in range(HEADS):
                                xs = X[:, b, hh * HID : (hh + 1) * HID]
                                wcol = ew[:, b * HEADS + hh : b * HEADS + hh + 1]
                                if hh < 2:
                                    nc.vector.tensor_scalar(
                                        wX[:, hh * HID : (hh + 1) * HID],
                                        xs, wcol, None, ALU.mult,
                                    )
                                else:
                                    nc.scalar.activation(
                                        wX[:, hh * HID : (hh + 1) * HID],
                                        xs, AF.Copy, scale=wcol,
                                    )
                            nc.tensor.matmul(
                                pout[:],
                                Sb_l[b][:],
                                wX[:],
                                start=(b == 0),
                                stop=(b == nblk - 1),
                            )
                            nc.tensor.matmul(
                                pz[:],
                                Sb_l[b][:],
                                ewr[:, b * HEADS : (b + 1) * HEADS],
                                start=(b == 0),
                                stop=(b == nblk - 1),
                            )
                        # divide by z, mean over heads, bias, relu, residual
                        zc = p2s.tile([128, HEADS], F32, tag="zc")
                        nc.vector.tensor_scalar(zc[:], pz[:], 1e-30, None, ALU.max)
                        zr = p2s.tile([128, HEADS], F32, tag="zr")
                        nc.vector.reciprocal(zr[:], zc[:])
                        nc.vector.tensor_scalar(zr[:], zr[:], 0.25, None, ALU.mult)
                        acc = p2s.tile([128, HID], F32, tag="acc")
                        nc.vector.tensor_scalar(
                            acc[:], pout[:, 0:HID], zr[:, 0:1], None, ALU.mult
                        )
                        for hh in range(1, HEADS):
                            nc.vector.scalar_tensor_tensor(
                                acc[:],
                                pout[:, hh * HID : (hh + 1) * HID],
                                zr[:, hh : hh + 1],
                                acc[:],
                                ALU.mult,
                                ALU.add,
                            )
                        nc.vector.tensor_add(acc[:], acc[:], btl[:, l, :])
                        nc.scalar.activation(acc[:], acc[:], AF.Relu)
                        hn = p2s.tile([128, HID], F32R, tag="hn")
                        nc.vector.tensor_add(
                            hn[:],
                            acc[:],
                            h_res[:, bass.ds(pid * TPC + tl, 1), :],
                        )
                        nc.sync.dma_start(cc_in[tl], hn[:])

                if dbg_stop == "p2":
                    break
                if dbg_stop in ("nocc", "sim"):
                    continue
                # ---- P3: allgather h ----
                nc.gpsimd.collective_compute(
                    "AllGather",
                    ALU.bypass,
                    replica_groups=[list(range(NCORE))],
                    ins=[cc_in[:, :, :].opt()],
                    outs=[ag_outs[l][:, :, :].opt()],
                )
                nc.sync.dma_start(
                    h_res[:, :, :],
                    ag_outs[l][:, :, :].rearrange("t p c -> p t c"),
                )

            with tc.tile_pool(name="hd", bufs=1) as hdp:
                hd = hdp.tile([128, 8 * HID], F32)
                for i in range(8):
                    nc.vector.tensor_copy(
                        hd[:, i * HID : (i + 1) * HID],
                        h_res[:, i * TPC_DUMP, :].bitcast(F32),
                    )
                nc.sync.dma_start(o_h[:], hd[:])
            if dbg_stop in ("p1", "p2", "p3", "sim"):
                return nc
            # ---- P4: graph mean pool + MLP ----
            with (
                tc.tile_pool(name="p4s", bufs=2) as p4s,
                tc.tile_pool(name="p4p", bufs=1, space="PSUM") as p4p,
            ):
                h_own = p4s.tile([128, TPC, HID], F32R)
                nc.sync.dma_start(
                    h_own[:, :, :], cc_in[:, :, :].rearrange("t p c -> p t c")
                )
                ppool = p4p.tile([32, HID], F32, tag="pool")
                for tl in range(TPC):
                    nc.tensor.matmul(
                        ppool[:],
                        btile[:, tl * 32 : (tl + 1) * 32],
                        h_own[:, tl, :],
                        start=(tl == 0),
                        stop=(tl == TPC - 1),
                    )
                pool_sb = p4s.tile([32, HID], F32)
                nc.vector.tensor_copy(pool_sb[:], ppool[:])
                nc.sync.dma_start(ar_in[:], pool_sb[:])
                nc.gpsimd.collective_compute(
                    "AllReduce",
                    ALU.add,
                    replica_groups=[list(range(NCORE))],
                    ins=[ar_in[:].opt()],
                    outs=[ar_out[:].opt()],
                )
                g_sb = p4s.tile([G, HID], F32)
                nc.sync.dma_start(g_sb[:], ar_out[:])

                def t_r(src_ap, pdim, fdim, tag):
                    """transpose + round to f32r: [pdim,fdim] -> [fdim,pdim]"""
                    ps = p4p.tile([fdim, pdim], F32, tag=tag + "p")
                    nc.tensor.transpose(ps[:], src_ap, ident[:pdim, :pdim])
                    sb = p4s.tile([fdim, pdim], F32R, tag=tag)
                    nc.vector.tensor_copy(sb[:], ps[:])
                    return sb

                W1 = p4s.tile([HID, 64], F32R)
                W2 = p4s.tile([64, 64], F32R)
                W3 = p4s.tile([64, 32], F32R)
                b1 = p4s.tile([32, 64], F32)
                b2 = p4s.tile([32, 64], F32)
                b3 = p4s.tile([32, 32], F32)
                nc.sync.dma_start(W1[:], t_W1[:])
                nc.sync.dma_start(W2[:], t_W2[:])
                nc.sync.dma_start(W3[:], t_W3[:])
                nc.sync.dma_start(b1[:], t_b1[:])
                nc.sync.dma_start(b2[:], t_b2[:])
                nc.sync.dma_start(b3[:], t_b3[:])

                gT = t_r(g_sb[:], G, HID, "gT")              # [128, 32]
                pm1 = p4p.tile([G, 64], F32, tag="m1")
                nc.tensor.matmul(pm1[:], gT[:], W1[:], start=True, stop=True)
                o1 = p4s.tile([G, 64], F32, tag="o1")
                nc.vector.tensor_add(o1[:], pm1[:], b1[:])
                nc.scalar.activation(o1[:], o1[:], AF.Relu)

                o1T = t_r(o1[:], G, 64, "o1T")               # [64, 32]
                pm2 = p4p.tile([G, 64], F32, tag="m2")
                nc.tensor.matmul(pm2[:], o1T[:], W2[:], start=True, stop=True)
                o2 = p4s.tile([G, 64], F32, tag="o2")
                nc.vector.tensor_add(o2[:], pm2[:], b2[:])
                nc.scalar.activation(o2[:], o2[:], AF.Relu)

                o2T = t_r(o2[:], G, 64, "o2T")               # [64, 32]
                pm3 = p4p.tile([G, 32], F32, tag="m3")
                nc.tensor.matmul(pm3[:], o2T[:], W3[:], start=True, stop=True)
                o3 = p4s.tile([G, 32], F32, tag="o3")
                nc.vector.tensor_add(o3[:], pm3[:], b3[:])
                nc.sync.dma_start(o_out[:], o3[:])
    return nc


_CACHE = {}
_LAST_NBLK = 11


def _get_program(nblk):
    if nblk not in _CACHE:
        nc = _build(nblk)
        _split_waits(nc)
        nc.compile()
        _CACHE[nblk] = nc
    return _CACHE[nblk]


def kernel(**inputs):
    import os

    inp = {k: np.asarray(v) for k, v in inputs.items()}
    prep = _preprocess(
        inp["x"].astype(np.float32),
        inp["edge_index"].astype(np.int64),
        inp["batch"].astype(np.int64),
        inp["gat_W"].astype(np.float32),
        inp["att_src"].astype(np.float32),
        inp["att_dst"].astype(np.float32),
    )
    nblk = prep["nblk"]
    global _LAST_NBLK
    _LAST_NBLK = nblk
    nc = _get_program(nblk)

    iota_row = np.tile(np.arange(128, dtype=np.float32)[None, :], (128, 1))
    iota_col = np.arange(128, dtype=np.float32)[:, None].copy()
    ident = np.eye(128, dtype=np.float32)
    btl = np.tile(
        inp["gat_b"].astype(np.float32)[:, None, :], (1, 128, 1)
    )  # [L,128,HID]
    b1t = np.tile(inp["b1"].astype(np.float32)[None, :], (32, 1))
    b2t = np.tile(inp["b2"].astype(np.float32)[None, :], (32, 1))
    b3t = np.tile(inp["b3"].astype(np.float32)[None, :], (32, 1))

    bt = mybir.dt.np(mybir.dt.bfloat16)
    shared = dict(
        xT=prep["xT"],
        iota_row=iota_row,
        iota_rowb=iota_row.astype(bt),
        iota_col=iota_col,
        ident=ident,
        identr=ident,
        Win=inp["W_in"].astype(np.float32),
        Wl=inp["gat_W"].astype(np.float32),
        WAl=prep["WA"],
        btl=btl,
        W1=inp["W1"].astype(np.float32),
        W2=inp["W2"].astype(np.float32),
        W3=inp["W3"].astype(np.float32),
        b1t=b1t,
        b2t=b2t,
        b3t=b3t,
    )
    in_maps = []
    for c in range(NCORE):
        m = dict(shared)
        m["gidx"] = prep["gidx"][c]
        m["dcols"] = prep["dcols"][c]
        m["dcolsb"] = prep["dcols"][c].astype(bt)
        m["btile"] = prep["btile"][c]
        in_maps.append(m)

    trace = bool(int(os.environ.get("KERNEL_TRACE", "0")))
    res = run_bass_kernel_spmd(
        nc, in_maps, core_ids=list(range(NCORE)), trace=trace
    )
    if trace and res.exec_time_ns is not None:
        print(f"HW exec time: {res.exec_time_ns} ns")
        kernel.last_exec_time_ns = res.exec_time_ns
        kernel.last_trace = res.instructions_and_trace
    return np.asarray(res.results[0]["out"], dtype=np.float32)


# revision 27
# speedup vs baseline: 1.3361x; 1.3361x over previous
"""GAT message-passing GNN on 8 Trainium2 NeuronCores (Bass/Tile).

Strategy: nodes are permuted (degree-balanced, round-robin over 160 tiles of
125 nodes) and partitioned across 8 cores (20 dst tiles each). Each layer:
every core redundantly computes xl = h @ W (and attention logits al = h @ WA)
for all nodes into a DRAM gather table; each core then processes its own dst
tiles: one dma_gather fetches xl[src] (+al_src) for all incident edges,
per-128-edge-block one-hot dst matrices are built with is_equal compares, and
the segment softmax + weighted scatter-add run as f32r matmuls accumulating in
PSUM (w = exp(leakyrelu(as+ad)) per edge; out = S^T @ (w*X); z = S^T @ w;
divide by z once per dst node). Updated node features are AllGathered each
layer. Final graph mean-pool is a one-hot matmul + AllReduce, then the MLP.
"""
import numpy as np

import concourse.bass as bass
import concourse.bacc as bacc
import concourse.mybir as mybir
import concourse.tile as tile
from concourse.bass_utils import run_bass_kernel_spmd

F32 = mybir.dt.float32
F32R = mybir.dt.float32r
BF16 = mybir.dt.bfloat16
I16 = mybir.dt.int16
AF = mybir.ActivationFunctionType
ALU = mybir.AluOpType

N, E, FIN, HID, HEADS, L, G = 20000, 200000, 20, 128, 4, 4, 32
NEG = 0.2
NCORE = 8
NT = 160            # global dst tiles
TPC = NT // NCORE   # 20 tiles per core
TILE_N = N // NT    # 125 real nodes per tile
PN = NT * 128       # padded node id space; PN = zero row
DROW = 640          # bf16 table row: 512 xl + 8 (4 f32 al_src) + pad (1280B, %256==0)
PADDST = 999.0
TPC_DUMP = 20  # dump tiles 0,20,40,... (first tile of each core)

_ZERO_WAIT_OPCODES = (
    "InstDMAGatherAnt",
    "InstDMAScatterAddAnt",
    "InstPartitionBroadcast",
    "InstPartitionAllReduce",
    "InstAPGather",
    "InstIndirectCopy",
    "InstSparseGather",
    "InstGatherTranspose",
)
_spill_counter = [0]


def _split_waits(nc, default_limit=1):
    """Spill excess semaphore waits onto preceding same-engine EventSemaphore
    instructions (walrus wait-slot limits: 0 for extended DMA ops, ~1+ else)."""
    for f in nc.m.functions:
        for bb in f.blocks:
            out = []
            changed = False
            for ins in bb.instructions:
                si = ins.sync_info
                waits = list(si.on_wait) if si is not None and si.on_wait else []
                tname = type(ins).__name__
                limit = default_limit
                if tname in _ZERO_WAIT_OPCODES:
                    limit = 0
                elif ins.engine == mybir.EngineType.Pool and tname in (
                    "InstDrain",
                    "InstNoOp",
                ):
                    limit = 0
                if len(waits) > limit:
                    changed = True
                    keep = waits[:limit] if limit else []
                    spill = waits[limit:] if limit else waits
                    while spill:
                        chunk, spill = spill[:1], spill[1:]
                        _spill_counter[0] += 1
                        nop = mybir.InstEventSemaphore(
                            name=f"waitspill-{_spill_counter[0]}"
                        )
                        nop.engine = ins.engine
                        nop.sync_info = mybir.SyncInfo(on_wait=chunk, on_update=[])
                        nc.register_instruction(nop, overwrite=True)
                        out.append(nop)
                    ins.sync_info = mybir.SyncInfo(
                        on_wait=keep, on_update=list(si.on_update) if si else []
                    )
                out.append(ins)
            if changed:
                bb.instructions[:] = out


def _preprocess(x, edge_index, batch, gat_W, att_src, att_dst):
    """Degree-balanced node permutation + per-core edge/tile data."""
    src = np.concatenate([edge_index[0], np.arange(N, dtype=np.int64)])
    dst = np.concatenate([edge_index[1], np.arange(N, dtype=np.int64)])
    indeg = np.bincount(dst, minlength=N)
    order = np.argsort(-indeg, kind="stable")
    new_id = np.empty(N, dtype=np.int64)
    ranks = np.arange(N)
    new_id[order] = (ranks % NT) * 128 + (ranks // NT)

    nsrc = new_id[src]
    ndst = new_id[dst]
    tile_e = ndst >> 7
    dloc = ndst & 127
    eorder = np.argsort(tile_e, kind="stable")
    tile_sorted = tile_e[eorder]
    nsrc_sorted = nsrc[eorder]
    dloc_sorted = dloc[eorder]
    starts = np.searchsorted(tile_sorted, np.arange(NT + 1))
    cnts = np.diff(starts)
    nblk = int(np.ceil(cnts.max() / 128))
    ET = nblk * 128

    gsrc = np.full((NT, ET), PN, dtype=np.int64)
    gdst = np.full((NT, ET), int(PADDST), dtype=np.int64)
    for t in range(NT):
        s, c = starts[t], cnts[t]
        gsrc[t, :c] = nsrc_sorted[s : s + c]
        gdst[t, :c] = dloc_sorted[s : s + c]

    # per-core arrays
    gidx = np.zeros((NCORE, 128, TPC * nblk * 8), dtype=np.int16)
    dcols = np.zeros((NCORE, 128, TPC * nblk), dtype=np.float32)
    for c in range(NCORE):
        for tl in range(TPC):
            t = c * TPC + tl
            wrap = gsrc[t].astype(np.int16).reshape(ET // 16, 16).T  # [16, ET/16]
            gidx[c, :, tl * nblk * 8 : (tl + 1) * nblk * 8] = np.tile(wrap, (8, 1))
            dcols[c, :, tl * nblk : (tl + 1) * nblk] = (
                gdst[t].reshape(nblk, 128).T.astype(np.float32)
            )

    # pooling matrix with 1/cnt folded in
    cnt = np.bincount(batch, minlength=G).astype(np.float32)
    cnt = np.maximum(cnt, 1.0)
    btile = np.zeros((NCORE, 128, TPC * 32), dtype=np.float32)
    inv = np.zeros(PN, dtype=np.int64)
    inv[new_id] = np.arange(N)  # new -> old (only valid slots)
    valid = np.zeros(PN, dtype=bool)
    valid[new_id] = True
    for c in range(NCORE):
        for tl in range(TPC):
            t = c * TPC + tl
            for p in range(TILE_N):
                nid = t * 128 + p
                if valid[nid]:
                    n_old = inv[nid]
                    g = batch[n_old]
                    btile[c, p, tl * 32 + g] = 1.0 / cnt[g]

    # permuted transposed input features
    xT = np.zeros((FIN, PN), dtype=np.float32)
    xT[:, new_id] = x.T

    # folded attention projections WA_l = W_l @ [A_src | A_dst]
    WA = np.zeros((L, HID, 2 * HEADS), dtype=np.float32)
    for l in range(L):
        A = np.zeros((HID * HEADS, 2 * HEADS), dtype=np.float64)
        for h in range(HEADS):
            A[h * HID : (h + 1) * HID, h] = att_src[l][h]
            A[h * HID : (h + 1) * HID, HEADS + h] = att_dst[l][h]
        WA[l] = (gat_W[l].astype(np.float64) @ A).astype(np.float32)

    return dict(gidx=gidx, dcols=dcols, btile=btile, xT=xT, WA=WA, nblk=nblk)


def _build(nblk, dbg_stop=None, dbg_layers=L, dbg_pad=0):
    ET = nblk * 128
    nc = bacc.Bacc("TRN2", target_bir_lowering=False, debug=False, num_devices=NCORE)

    t_xT = nc.dram_tensor("xT", [FIN, PN], F32, kind="ExternalInput")
    t_gidx = nc.dram_tensor("gidx", [128, TPC * nblk * 8], I16, kind="ExternalInput")
    t_dcols = nc.dram_tensor("dcols", [128, TPC * nblk], F32, kind="ExternalInput")
    t_dcolsb = nc.dram_tensor("dcolsb", [128, TPC * nblk], BF16, kind="ExternalInput")
    t_iota_rowb = nc.dram_tensor("iota_rowb", [128, 128], BF16, kind="ExternalInput")
    t_btile = nc.dram_tensor("btile", [128, TPC * 32], F32R, kind="ExternalInput")
    t_iota_row = nc.dram_tensor("iota_row", [128, 128], F32, kind="ExternalInput")
    t_iota_col = nc.dram_tensor("iota_col", [128, 1], F32, kind="ExternalInput")
    t_ident = nc.dram_tensor("ident", [128, 128], F32, kind="ExternalInput")
    t_identr = nc.dram_tensor("identr", [128, 128], F32R, kind="ExternalInput")
    t_Win = nc.dram_tensor("Win", [FIN, HID], F32, kind="ExternalInput")
    t_Wl = nc.dram_tensor("Wl", [L, HID, HEADS * HID], F32R, kind="ExternalInput")
    t_WAl = nc.dram_tensor("WAl", [L, HID, 2 * HEADS], F32R, kind="ExternalInput")
    t_bcol = nc.dram_tensor("bcol", [L, HID, 1], F32, kind="ExternalInput")
    t_W1 = nc.dram_tensor("W1", [HID, 64], F32R, kind="ExternalInput")
    t_W2 = nc.dram_tensor("W2", [64, 64], F32R, kind="ExternalInput")
    t_W3 = nc.dram_tensor("W3", [64, 32], F32R, kind="ExternalInput")
    t_b1 = nc.dram_tensor("b1t", [32, 64], F32, kind="ExternalInput")
    t_b2 = nc.dram_tensor("b2t", [32, 64], F32, kind="ExternalInput")
    t_b3 = nc.dram_tensor("b3t", [32, 32], F32, kind="ExternalInput")
    o_out = nc.dram_tensor("out", [G, 32], F32, kind="ExternalOutput")
    o_h = nc.dram_tensor("hdump", [128, 8 * HID], F32, kind="ExternalOutput")

    table = nc.dram_tensor("table", [PN + 1, DROW], BF16)

    with tile.TileContext(nc) as tc:
        with (
            tc.tile_pool(name="const", bufs=1) as cpool,
            tc.tile_pool(name="persist", bufs=1) as hpool,
            tc.tile_pool(name="dram", bufs=1, space="DRAM") as dpool,
        ):
            # ---- constants to SBUF ----
            iota_row = cpool.tile([128, 128], F32)
            iota_col = cpool.tile([128, 1], F32)
            ident = cpool.tile([128, 128], F32)
            identr = cpool.tile([128, 128], F32R)
            Win = cpool.tile([FIN, HID], F32)
            Wl = cpool.tile([128, L, HEADS * HID], F32R)
            WAl = cpool.tile([128, L, 2 * HEADS], F32R)
            bcol = cpool.tile([HID, L], F32)
            gidx = cpool.tile([128, TPC * nblk * 8], I16)
            dcols = cpool.tile([128, TPC * nblk], F32)
            dcolsb = cpool.tile([128, TPC * nblk], BF16)
            iota_rowb = cpool.tile([128, 128], BF16)
            btile = cpool.tile([128, TPC * 32], F32R)
            nc.sync.dma_start(iota_row[:], t_iota_row[:])
            nc.sync.dma_start(iota_col[:], t_iota_col[:])
            nc.sync.dma_start(ident[:], t_ident[:])
            nc.sync.dma_start(identr[:], t_identr[:])
            nc.sync.dma_start(Win[:], t_Win[:])
            for l in range(L):
                nc.sync.dma_start(Wl[:, l, :], t_Wl[l])
                nc.sync.dma_start(WAl[:, l, :], t_WAl[l])
                nc.sync.dma_start(bcol[:, l : l + 1], t_bcol[l])
            nc.gpsimd.dma_start(gidx[:], t_gidx[:])
            nc.sync.dma_start(dcols[:], t_dcols[:])
            nc.sync.dma_start(dcolsb[:], t_dcolsb[:])
            nc.sync.dma_start(iota_rowb[:], t_iota_rowb[:])
            nc.sync.dma_start(btile[:], t_btile[:])

            # persistent node features [p, tile, c]
            h_resT = hpool.tile([128, NT * 128], F32R)
            al_dst_all = hpool.tile([128, NT, HEADS], BF16)
            al_dst_own = hpool.tile([128, TPC, HEADS], BF16)

            # zero row of the gather table
            with tc.tile_pool(name="zr", bufs=1) as zpool:
                zrow = zpool.tile([1, DROW], BF16)
                nc.vector.memset(zrow[:], 0.0)
                nc.sync.dma_start(table[PN : PN + 1, :], zrow[:])

            # collective buffers
            cc_in = dpool.tile([TPC, 128, HID], F32R)
            ag_outs = [
                dpool.tile([NT, 128, HID], F32R, addr_space="Shared", name=f"ag{i}")
                for i in range(L)
            ]
            ar_in = dpool.tile([G, HID], F32)
            ar_out = dpool.tile([G, HID], F32, addr_space="Shared")

            pid = nc.vector.partition_id()

            if dbg_pad:
                with tc.tile_pool(name="padp", bufs=2) as padp:
                    pa = padp.tile([1, 16], F32, tag="pa")
                    nc.vector.memset(pa[:], 0.0)
                    for _ in range(dbg_pad):
                        pb = padp.tile([1, 16], F32, tag="pa")
                        nc.vector.tensor_copy(pb[:], pa[:])
                        pa = pb
            # ---- phase 0: h0 = relu(x @ Win) ----
            with (
                tc.tile_pool(name="p0s", bufs=3) as p0s,
                tc.tile_pool(name="p0p", bufs=2, space="PSUM") as p0p,
            ):
                for t0 in range(0, NT, 8):
                    xt = p0s.tile([FIN, 8, 128], F32, tag="xt")
                    nc.sync.dma_start(
                        xt[:], t_xT[:, t0 * 128 : (t0 + 8) * 128]
                    )
                    for j in range(8):
                        t = t0 + j
                        ph = p0p.tile([HID, 128], F32, tag="ph")
                        nc.tensor.matmul(
                            ph[:], Win[:], xt[:, j, :], start=True, stop=True
                        )
                        nc.scalar.activation(
                            h_resT[:, t * 128 : (t + 1) * 128], ph[:], AF.Relu
                        )

            for l in range(dbg_layers):
                # ---- P1: xl/al for all nodes -> gather table ----
                with (
                    tc.tile_pool(name="p1s", bufs=3) as p1s,
                    tc.tile_pool(name="p1p", bufs=2, space="PSUM") as p1p,
                ):
                    for t0 in range(0, NT, 4):
                        stage = p1s.tile([128, 4, DROW], BF16, tag="st")
                        for j in range(4):
                            t = t0 + j
                            hT = h_resT[:, t * 128 : (t + 1) * 128]
                            pxl = p1p.tile([128, HEADS * HID], F32, tag="xl")
                            nc.tensor.matmul(
                                pxl[:], hT, Wl[:, l, :], start=True, stop=True
                            )
                            pal = p1p.tile([128, 2 * HEADS], F32, tag="al")
                            nc.tensor.matmul(
                                pal[:], hT, WAl[:, l, :], start=True, stop=True
                            )
                            if t % 3 == 0:
                                nc.vector.tensor_copy(stage[:, j, 0:512], pxl[:])
                            else:
                                nc.scalar.copy(stage[:, j, 0:512], pxl[:])
                            nc.vector.tensor_copy(
                                stage[:, j, 512:520].bitcast(F32), pal[:, 0:HEADS]
                            )
                            nc.vector.tensor_copy(
                                al_dst_all[:, t, :], pal[:, HEADS : 2 * HEADS]
                            )
                        nc.sync.dma_start(
                            table[t0 * 128 : (t0 + 4) * 128, 0:520].rearrange(
                                "(j p) c -> p j c", j=4
                            ),
                            stage[:, :, 0:520],
                        )
                    # own slice of al_dst (core-dependent via register offset)
                    nc.vector.tensor_copy(
                        al_dst_own[:, :, :],
                        al_dst_all[:, bass.ds(pid * TPC, TPC), :],
                    )

                if dbg_stop == "p1":
                    break
                # ---- P2: per own dst tile: gather + attention + scatter ----
                with (
                    tc.tile_pool(name="p2s", bufs=2) as p2s,
                    tc.tile_pool(name="p2p", bufs=2, space="PSUM") as p2p,
                ):
                    for tl in range(TPC):
                        X = p2s.tile([128, nblk, DROW], BF16, tag="X")
                        for g0 in range(0, nblk, 8):
                            g1 = min(g0 + 8, nblk)
                            nc.gpsimd.dma_gather(
                                X[:, g0:g1, :],
                                table[:],
                                gidx[
                                    :,
                                    tl * nblk * 8 + g0 * 8 : tl * nblk * 8 + g1 * 8,
                                ],
                                (g1 - g0) * 128,
                                (g1 - g0) * 128,
                                DROW,
                            )
                        pad = p2p.tile([128, nblk * HEADS], F32, tag="ad", bufs=1)
                        pout = p2p.tile([128, HEADS * HID], F32, tag="out")
                        pz = p2p.tile([128, HEADS], F32, tag="z")
                        Sb_l = []
                        for b in range(nblk):
                            dcol = dcols[:, tl * nblk + b : tl * nblk + b + 1]
                            drows = p2p.tile([128, 128], F32, tag="dr")
                            nc.tensor.transpose(
                                drows[:], dcol.to_broadcast([128, 128]), ident[:]
                            )
                            Sb = p2s.tile([128, 128], BF16, tag=f"Sb{b % 2}")
                            SbT = p2s.tile([128, 128], BF16, tag=f"SbT{b % 2}")
                            nc.vector.tensor_scalar(
                                Sb[:], iota_rowb[:], dcol, None, ALU.is_equal
                            )
                            nc.vector.tensor_scalar(
                                SbT[:], drows[:], iota_col[:], None, ALU.is_equal
                            )
                            nc.tensor.matmul(
                                pad[:, b * HEADS : (b + 1) * HEADS],
                                SbT[:],
                                al_dst_own[:, tl, :],
                                start=True,
                                stop=True,
                            )
                            Sb_l.append(Sb)
                        # attention weights for the whole tile
                        ew = p2s.tile([128, nblk * HEADS], F32, tag="ew")
                        nc.vector.tensor_add(
                            ew[:],
                            X[:, :, 512:520].bitcast(F32),
                            pad[:],
                        )
                        nc.vector.scalar_tensor_tensor(
                            ew[:], ew[:], NEG, ew[:], ALU.mult, ALU.max
                        )
                        nc.scalar.activation(ew[:], ew[:], AF.Exp)
                        ewr = p2s.tile([128, nblk * HEADS], BF16, tag="ewr")
                        nc.vector.tensor_copy(ewr[:], ew[:])
                        for b in range(nblk):
                            wX = p2s.tile([128, HEADS * HID], BF16, tag=f"wX{b % 2}")
                            for hh in range(HEADS):
                                xs = X[:, b, hh * HID : (hh + 1) * HID]
                                wcol = ew[:, b * HEADS + hh : b * HEADS + hh + 1]
                                if hh < 2:
                                    nc.vector.tensor_scalar(
                                        wX[:, hh * HID : (hh + 1) * HID],
                                        xs, wcol, None, ALU.mult,
                                    )
                                else:
                                    nc.scalar.activation(
                                        wX[:, hh * HID : (hh + 1) * HID],
                                        xs, AF.Copy, scale=wcol,
                                    )
                            nc.tensor.matmul(
                                pout[:],
                                Sb_l[b][:],
                                wX[:],
                                start=(b == 0),
                                stop=(b == nblk - 1),
                            )
                            nc.tensor.matmul(
                                pz[:],
                                Sb_l[b][:],
                                ewr[:, b * HEADS : (b + 1) * HEADS],
                                start=(b == 0),
                                stop=(b == nblk - 1),
                            )
                        # divide by z, mean over heads, bias, relu, residual
                        zc = p2s.tile([128, HEADS], F32, tag="zc")
                        nc.vector.tensor_scalar(zc[:], pz[:], 1e-30, None, ALU.max)
                        zr = p2s.tile([128, HEADS], F32, tag="zr")
                        nc.vector.reciprocal(zr[:], zc[:])
                        nc.vector.tensor_scalar(zr[:], zr[:], 0.25, None, ALU.mult)
                        acc = p2s.tile([128, HID], F32, tag="acc")
                        nc.vector.tensor_scalar(
                            acc[:], pout[:, 0:HID], zr[:, 0:1], None, ALU.mult
                        )
                        for hh in range(1, HEADS):
                            nc.vector.scalar_tensor_tensor(
                                acc[:],
                                pout[:, hh * HID : (hh + 1) * HID],
                                zr[:, hh : hh + 1],
                                acc[:],
                                ALU.mult,
                                ALU.add,
                            )
                        accT = p2p.tile([HID, 128], F32, tag="accT", bufs=1)
                        nc.tensor.transpose(accT[:], acc[:], ident[:])
                        hr = p2s.tile([HID, 128], F32R, tag="hr")
                        nc.scalar.activation(
                            hr[:], accT[:], AF.Relu, bias=bcol[:, l : l + 1]
                        )
                        hn = p2s.tile([HID, 128], F32R, tag="hn")
                        nc.vector.tensor_add(
                            hn[:],
                            hr[:],
                            h_resT[:, bass.ds((pid * TPC + tl) * 128, 128)],
                        )
                        nc.sync.dma_start(cc_in[tl], hn[:])

                if dbg_stop == "p2":
                    break
                if dbg_stop in ("nocc", "sim"):
                    continue
                # ---- P3: allgather h ----
                nc.gpsimd.collective_compute(
                    "AllGather",
                    ALU.bypass,
                    replica_groups=[list(range(NCORE))],
                    ins=[cc_in[:, :, :].opt()],
                    outs=[ag_outs[l][:, :, :].opt()],
                )
                nc.sync.dma_start(
                    h_resT[:, :].rearrange("c (t n) -> c t n", t=NT),
                    ag_outs[l][:, :, :].rearrange("t c n -> c t n"),
                )

            with tc.tile_pool(name="hd", bufs=1) as hdp:
                hd = hdp.tile([128, 8 * HID], F32)
                for i in range(8):
                    nc.vector.tensor_copy(
                        hd[:, i * HID : (i + 1) * HID],
                        h_resT[
                            :, i * TPC_DUMP * 128 : (i * TPC_DUMP + 1) * 128
                        ].bitcast(F32),
                    )
                nc.sync.dma_start(o_h[:], hd[:])
            if dbg_stop in ("p1", "p2", "p3", "sim"):
                return nc
            # ---- P4: graph mean pool + MLP ----
            with (
                tc.tile_pool(name="p4s", bufs=2) as p4s,
                tc.tile_pool(name="p4p", bufs=1, space="PSUM") as p4p,
            ):
                h_ownT = p4s.tile([128, TPC, 128], F32R)
                nc.sync.dma_start(
                    h_ownT[:, :, :], cc_in[:, :, :].rearrange("t c n -> c t n")
                )
                ppool = p4p.tile([32, HID], F32, tag="pool")
                for tl in range(TPC):
                    hnm_ps = p4p.tile([128, HID], F32R, tag="hnm")
                    nc.tensor.transpose(
                        hnm_ps[:], h_ownT[:, tl, :], identr[:]
                    )
                    hnm = p4s.tile([128, HID], F32R, tag="hnm")
                    nc.vector.tensor_copy(hnm[:], hnm_ps[:])
                    nc.tensor.matmul(
                        ppool[:],
                        btile[:, tl * 32 : (tl + 1) * 32],
                        hnm[:],
                        start=(tl == 0),
                        stop=(tl == TPC - 1),
                    )
                pool_sb = p4s.tile([32, HID], F32)
                nc.vector.tensor_copy(pool_sb[:], ppool[:])
                nc.sync.dma_start(ar_in[:], pool_sb[:])
                nc.gpsimd.collective_compute(
                    "AllReduce",
                    ALU.add,
                    replica_groups=[list(range(NCORE))],
                    ins=[ar_in[:].opt()],
                    outs=[ar_out[:].opt()],
                )
                g_sb = p4s.tile([G, HID], F32)
                nc.sync.dma_start(g_sb[:], ar_out[:])

                def t_r(src_ap, pdim, fdim, tag):
                    """transpose + round to f32r: [pdim,fdim] -> [fdim,pdim]"""
                    ps = p4p.tile([fdim, pdim], F32, tag=tag + "p")
                    nc.tensor.transpose(ps[:], src_ap, ident[:pdim, :pdim])
                    sb = p4s.tile([fdim, pdim], F32R, tag=tag)
                    nc.vector.tensor_copy(sb[:], ps[:])
                    return sb

                W1 = p4s.tile([HID, 64], F32R)
                W2 = p4s.tile([64, 64], F32R)
                W3 = p4s.tile([64, 32], F32R)
                b1 = p4s.tile([32, 64], F32)
                b2 = p4s.tile([32, 64], F32)
                b3 = p4s.tile([32, 32], F32)
                nc.sync.dma_start(W1[:], t_W1[:])
                nc.sync.dma_start(W2[:], t_W2[:])
                nc.sync.dma_start(W3[:], t_W3[:])
                nc.sync.dma_start(b1[:], t_b1[:])
                nc.sync.dma_start(b2[:], t_b2[:])
                nc.sync.dma_start(b3[:], t_b3[:])

                gT = t_r(g_sb[:], G, HID, "gT")              # [128, 32]
                pm1 = p4p.tile([G, 64], F32, tag="m1")
                nc.tensor.matmul(pm1[:], gT[:], W1[:], start=True, stop=True)
                o1 = p4s.tile([G, 64], F32, tag="o1")
                nc.vector.tensor_add(o1[:], pm1[:], b1[:])
                nc.scalar.activation(o1[:], o1[:], AF.Relu)

                o1T = t_r(o1[:], G, 64, "o1T")               # [64, 32]
                pm2 = p4p.tile([G, 64], F32, tag="m2")
                nc.tensor.matmul(pm2[:], o1T[:], W2[:], start=True, stop=True)
                o2 = p4s.tile([G, 64], F32, tag="o2")
                nc.vector.tensor_add(o2[:], pm2[:], b2[:])
                nc.scalar.activation(o2[:], o2[:], AF.Relu)

                o2T = t_r(o2[:], G, 64, "o2T")               # [64, 32]
                pm3 = p4p.tile([G, 32], F32, tag="m3")
                nc.tensor.matmul(pm3[:], o2T[:], W3[:], start=True, stop=True)
                o3 = p4s.tile([G, 32], F32, tag="o3")
                nc.vector.tensor_add(o3[:], pm3[:], b3[:])
                nc.sync.dma_start(o_out[:], o3[:])
    return nc


_CACHE = {}
_LAST_NBLK = 11


def _get_program(nblk):
    if nblk not in _CACHE:
        nc = _build(nblk)
        _split_waits(nc)
        nc.compile()
        _CACHE[nblk] = nc
    return _CACHE[nblk]


def kernel(**inputs):
    import os

    inp = {k: np.asarray(v) for k, v in inputs.items()}
    prep = _preprocess(
        inp["x"].astype(np.float32),
        inp["edge_index"].astype(np.int64),
        inp["batch"].astype(np.int64),
        inp["gat_W"].astype(np.float32),
        inp["att_src"].astype(np.float32),
        inp["att_dst"].astype(np.float32),
    )
    nblk = prep["nblk"]
    global _LAST_NBLK
    _LAST_NBLK = nblk
    nc = _get_program(nblk)

    iota_row = np.tile(np.arange(128, dtype=np.float32)[None, :], (128, 1))
    iota_col = np.arange(128, dtype=np.float32)[:, None].copy()
    ident = np.eye(128, dtype=np.float32)
    bcol = inp["gat_b"].astype(np.float32)[:, :, None]  # [L,HID,1]
    b1t = np.tile(inp["b1"].astype(np.float32)[None, :], (32, 1))
    b2t = np.tile(inp["b2"].astype(np.float32)[None, :], (32, 1))
    b3t = np.tile(inp["b3"].astype(np.float32)[None, :], (32, 1))

    bt = mybir.dt.np(mybir.dt.bfloat16)
    shared = dict(
        xT=prep["xT"],
        iota_row=iota_row,
        iota_rowb=iota_row.astype(bt),
        iota_col=iota_col,
        ident=ident,
        identr=ident,
        Win=inp["W_in"].astype(np.float32),
        Wl=inp["gat_W"].astype(np.float32),
        WAl=prep["WA"],
        bcol=bcol,
        W1=inp["W1"].astype(np.float32),
        W2=inp["W2"].astype(np.float32),
        W3=inp["W3"].astype(np.float32),
        b1t=b1t,
        b2t=b2t,
        b3t=b3t,
    )
    in_maps = []
    for c in range(NCORE):
        m = dict(shared)
        m["gidx"] = prep["gidx"][c]
        m["dcols"] = prep["dcols"][c]
        m["dcolsb"] = prep["dcols"][c].astype(bt)
        m["btile"] = prep["btile"][c]
        in_maps.append(m)

    trace = bool(int(os.environ.get("KERNEL_TRACE", "0")))
    last_exc = None
    for attempt in range(3):
        try:
            res = run_bass_kernel_spmd(
                nc, in_maps, core_ids=list(range(NCORE)), trace=trace
            )
            break
        except Exception as exc:  # transient device-unrecoverable after crashes
            last_exc = exc
            import time as _time

            _time.sleep(15)
    else:
        raise last_exc
    if trace and res.exec_time_ns is not None:
        print(f"HW exec time: {res.exec_time_ns} ns")
        kernel.last_exec_time_ns = res.exec_time_ns
        kernel.last_trace = res.instructions_and_trace
    return np.asarray(res.results[0]["out"], dtype=np.float32)


# revision 32
# speedup vs baseline: 1.4288x; 1.0693x over previous
"""GAT message-passing GNN on 8 Trainium2 NeuronCores (Bass/Tile).

Strategy: nodes are permuted (degree-balanced, round-robin over 160 tiles of
125 nodes) and partitioned across 8 cores (20 dst tiles each). Each layer:
every core redundantly computes xl = h @ W (and attention logits al = h @ WA)
for all nodes into a DRAM gather table; each core then processes its own dst
tiles: one dma_gather fetches xl[src] (+al_src) for all incident edges,
per-128-edge-block one-hot dst matrices are built with is_equal compares, and
the segment softmax + weighted scatter-add run as f32r matmuls accumulating in
PSUM (w = exp(leakyrelu(as+ad)) per edge; out = S^T @ (w*X); z = S^T @ w;
divide by z once per dst node). Updated node features are AllGathered each
layer. Final graph mean-pool is a one-hot matmul + AllReduce, then the MLP.
"""
import numpy as np

import concourse.bass as bass
import concourse.bacc as bacc
import concourse.mybir as mybir
import concourse.tile as tile
from concourse.bass_utils import run_bass_kernel_spmd

F32 = mybir.dt.float32
F32R = mybir.dt.float32r
BF16 = mybir.dt.bfloat16
I16 = mybir.dt.int16
AF = mybir.ActivationFunctionType
ALU = mybir.AluOpType

N, E, FIN, HID, HEADS, L, G = 20000, 200000, 20, 128, 4, 4, 32
NEG = 0.2
NCORE = 8
NT = 160            # global dst tiles
TPC = NT // NCORE   # 20 tiles per core
TILE_N = N // NT    # 125 real nodes per tile
PN = NT * 128       # padded node id space; PN = zero row
DROW = 640          # bf16 table row: 512 xl + 8 (4 f32 al_src) + pad (1280B, %256==0)
PADDST = 999.0
TPC_DUMP = 20  # dump tiles 0,20,40,... (first tile of each core)

_ZERO_WAIT_OPCODES = (
    "InstDMAGatherAnt",
    "InstDMAScatterAddAnt",
    "InstPartitionBroadcast",
    "InstPartitionAllReduce",
    "InstAPGather",
    "InstIndirectCopy",
    "InstSparseGather",
    "InstGatherTranspose",
)
_spill_counter = [0]


def _split_waits(nc, default_limit=1):
    """Spill excess semaphore waits onto preceding same-engine EventSemaphore
    instructions (walrus wait-slot limits: 0 for extended DMA ops, ~1+ else)."""
    for f in nc.m.functions:
        for bb in f.blocks:
            out = []
            changed = False
            for ins in bb.instructions:
                si = ins.sync_info
                waits = list(si.on_wait) if si is not None and si.on_wait else []
                tname = type(ins).__name__
                limit = default_limit
                if tname in _ZERO_WAIT_OPCODES:
                    limit = 0
                elif ins.engine == mybir.EngineType.Pool and tname in (
                    "InstDrain",
                    "InstNoOp",
                ):
                    limit = 0
                if len(waits) > limit:
                    changed = True
                    keep = waits[:limit] if limit else []
                    spill = waits[limit:] if limit else waits
                    while spill:
                        chunk, spill = spill[:1], spill[1:]
                        _spill_counter[0] += 1
                        nop = mybir.InstEventSemaphore(
                            name=f"waitspill-{_spill_counter[0]}"
                        )
                        nop.engine = ins.engine
                        nop.sync_info = mybir.SyncInfo(on_wait=chunk, on_update=[])
                        nc.register_instruction(nop, overwrite=True)
                        out.append(nop)
                    ins.sync_info = mybir.SyncInfo(
                        on_wait=keep, on_update=list(si.on_update) if si else []
                    )
                out.append(ins)
            if changed:
                bb.instructions[:] = out


def _preprocess(x, edge_index, batch, gat_W, att_src, att_dst):
    """Degree-balanced node permutation + per-core edge/tile data."""
    src = np.concatenate([edge_index[0], np.arange(N, dtype=np.int64)])
    dst = np.concatenate([edge_index[1], np.arange(N, dtype=np.int64)])
    indeg = np.bincount(dst, minlength=N)
    order = np.argsort(-indeg, kind="stable")
    new_id = np.empty(N, dtype=np.int64)
    ranks = np.arange(N)
    new_id[order] = (ranks % NT) * 128 + (ranks // NT)

    nsrc = new_id[src]
    ndst = new_id[dst]
    tile_e = ndst >> 7
    dloc = ndst & 127
    eorder = np.argsort(tile_e, kind="stable")
    tile_sorted = tile_e[eorder]
    nsrc_sorted = nsrc[eorder]
    dloc_sorted = dloc[eorder]
    starts = np.searchsorted(tile_sorted, np.arange(NT + 1))
    cnts = np.diff(starts)
    nblk = int(np.ceil(cnts.max() / 128))
    ET = nblk * 128

    gsrc = np.full((NT, ET), PN, dtype=np.int64)
    gdst = np.full((NT, ET), int(PADDST), dtype=np.int64)
    for t in range(NT):
        s, c = starts[t], cnts[t]
        gsrc[t, :c] = nsrc_sorted[s : s + c]
        gdst[t, :c] = dloc_sorted[s : s + c]

    # per-core arrays
    gidx = np.zeros((NCORE, 128, TPC * nblk * 8), dtype=np.int16)
    dcols = np.zeros((NCORE, 128, TPC * nblk), dtype=np.float32)
    for c in range(NCORE):
        for tl in range(TPC):
            t = c * TPC + tl
            wrap = gsrc[t].astype(np.int16).reshape(ET // 16, 16).T  # [16, ET/16]
            gidx[c, :, tl * nblk * 8 : (tl + 1) * nblk * 8] = np.tile(wrap, (8, 1))
            dcols[c, :, tl * nblk : (tl + 1) * nblk] = (
                gdst[t].reshape(nblk, 128).T.astype(np.float32)
            )

    # pooling matrix with 1/cnt folded in
    cnt = np.bincount(batch, minlength=G).astype(np.float32)
    cnt = np.maximum(cnt, 1.0)
    btile = np.zeros((NCORE, 128, TPC * 32), dtype=np.float32)
    inv = np.zeros(PN, dtype=np.int64)
    inv[new_id] = np.arange(N)  # new -> old (only valid slots)
    valid = np.zeros(PN, dtype=bool)
    valid[new_id] = True
    for c in range(NCORE):
        for tl in range(TPC):
            t = c * TPC + tl
            for p in range(TILE_N):
                nid = t * 128 + p
                if valid[nid]:
                    n_old = inv[nid]
                    g = batch[n_old]
                    btile[c, p, tl * 32 + g] = 1.0 / cnt[g]

    # permuted transposed input features
    xT = np.zeros((FIN, PN), dtype=np.float32)
    xT[:, new_id] = x.T

    # folded attention projections WA_l = W_l @ [A_src | A_dst]
    WA = np.zeros((L, HID, 2 * HEADS), dtype=np.float32)
    for l in range(L):
        A = np.zeros((HID * HEADS, 2 * HEADS), dtype=np.float64)
        for h in range(HEADS):
            A[h * HID : (h + 1) * HID, h] = att_src[l][h]
            A[h * HID : (h + 1) * HID, HEADS + h] = att_dst[l][h]
        WA[l] = (gat_W[l].astype(np.float64) @ A).astype(np.float32)

    return dict(gidx=gidx, dcols=dcols, btile=btile, xT=xT, WA=WA, nblk=nblk)


def _build(nblk, dbg_stop=None, dbg_layers=L, dbg_pad=0):
    ET = nblk * 128
    nc = bacc.Bacc("TRN2", target_bir_lowering=False, debug=False, num_devices=NCORE)

    t_xT = nc.dram_tensor("xT", [FIN, PN], F32, kind="ExternalInput")
    t_gidx = nc.dram_tensor("gidx", [128, TPC * nblk * 8], I16, kind="ExternalInput")
    t_dcols = nc.dram_tensor("dcols", [128, TPC * nblk], F32, kind="ExternalInput")
    t_dcolsb = nc.dram_tensor("dcolsb", [128, TPC * nblk], BF16, kind="ExternalInput")
    t_iota_rowb = nc.dram_tensor("iota_rowb", [128, 128], BF16, kind="ExternalInput")
    t_btile = nc.dram_tensor("btile", [128, TPC * 32], F32R, kind="ExternalInput")
    t_iota_row = nc.dram_tensor("iota_row", [128, 128], F32, kind="ExternalInput")
    t_iota_col = nc.dram_tensor("iota_col", [128, 1], F32, kind="ExternalInput")
    t_ident = nc.dram_tensor("ident", [128, 128], F32, kind="ExternalInput")
    t_identr = nc.dram_tensor("identr", [128, 128], F32R, kind="ExternalInput")
    t_Win = nc.dram_tensor("Win", [FIN, HID], F32, kind="ExternalInput")
    t_Wl = nc.dram_tensor("Wl", [L, HID, HEADS * HID], F32R, kind="ExternalInput")
    t_WAl = nc.dram_tensor("WAl", [L, HID, 2 * HEADS], F32R, kind="ExternalInput")
    t_bcol = nc.dram_tensor("bcol", [L, HID, 1], F32, kind="ExternalInput")
    t_W1 = nc.dram_tensor("W1", [HID, 64], F32R, kind="ExternalInput")
    t_W2 = nc.dram_tensor("W2", [64, 64], F32R, kind="ExternalInput")
    t_W3 = nc.dram_tensor("W3", [64, 32], F32R, kind="ExternalInput")
    t_b1 = nc.dram_tensor("b1t", [32, 64], F32, kind="ExternalInput")
    t_b2 = nc.dram_tensor("b2t", [32, 64], F32, kind="ExternalInput")
    t_b3 = nc.dram_tensor("b3t", [32, 32], F32, kind="ExternalInput")
    o_out = nc.dram_tensor("out", [G, 32], F32, kind="ExternalOutput")
    o_h = nc.dram_tensor("hdump", [128, 8 * HID], F32, kind="ExternalOutput")

    table = nc.dram_tensor("table", [PN + 1, DROW], BF16)

    with tile.TileContext(nc) as tc:
        with (
            tc.tile_pool(name="const", bufs=1) as cpool,
            tc.tile_pool(name="persist", bufs=1) as hpool,
            tc.tile_pool(name="dram", bufs=1, space="DRAM") as dpool,
        ):
            # ---- constants to SBUF ----
            iota_row = cpool.tile([128, 128], F32)
            iota_col = cpool.tile([128, 1], F32)
            ident = cpool.tile([128, 128], F32)
            identr = cpool.tile([128, 128], F32R)
            Win = cpool.tile([FIN, HID], F32)
            Wl = cpool.tile([128, L, HEADS * HID], F32R)
            WAl = cpool.tile([128, L, 2 * HEADS], F32R)
            bcol = cpool.tile([HID, L], F32)
            gidx = cpool.tile([128, TPC * nblk * 8], I16)
            dcols = cpool.tile([128, TPC * nblk], F32)
            dcolsb = cpool.tile([128, TPC * nblk], BF16)
            iota_rowb = cpool.tile([128, 128], BF16)
            btile = cpool.tile([128, TPC * 32], F32R)
            nc.sync.dma_start(iota_row[:], t_iota_row[:])
            nc.sync.dma_start(iota_col[:], t_iota_col[:])
            nc.sync.dma_start(ident[:], t_ident[:])
            nc.sync.dma_start(identr[:], t_identr[:])
            nc.sync.dma_start(Win[:], t_Win[:])
            for l in range(L):
                nc.sync.dma_start(Wl[:, l, :], t_Wl[l])
                nc.sync.dma_start(WAl[:, l, :], t_WAl[l])
                nc.sync.dma_start(bcol[:, l : l + 1], t_bcol[l])
            nc.gpsimd.dma_start(gidx[:], t_gidx[:])
            nc.sync.dma_start(dcols[:], t_dcols[:])
            nc.sync.dma_start(dcolsb[:], t_dcolsb[:])
            nc.sync.dma_start(iota_rowb[:], t_iota_rowb[:])
            nc.sync.dma_start(btile[:], t_btile[:])

            # persistent node features [p, tile, c]
            h_resT = hpool.tile([128, NT * 128], F32R)
            al_dst_all = hpool.tile([128, NT, HEADS], BF16)
            al_dst_own = hpool.tile([128, TPC, HEADS], BF16)

            # zero row of the gather table
            with tc.tile_pool(name="zr", bufs=1) as zpool:
                zrow = zpool.tile([1, DROW], BF16)
                nc.vector.memset(zrow[:], 0.0)
                nc.sync.dma_start(table[PN : PN + 1, :], zrow[:])

            # collective buffers
            cc_in = dpool.tile([TPC, 128, HID], F32R)
            ag_outs = [
                dpool.tile([NT, 128, HID], F32R, addr_space="Shared", name=f"ag{i}")
                for i in range(L)
            ]
            ar_in = dpool.tile([G, HID], F32)
            ar_out = dpool.tile([G, HID], F32, addr_space="Shared")

            pid = nc.vector.partition_id()

            if dbg_pad:
                with tc.tile_pool(name="padp", bufs=2) as padp:
                    pa = padp.tile([1, 16], F32, tag="pa")
                    nc.vector.memset(pa[:], 0.0)
                    for _ in range(dbg_pad):
                        pb = padp.tile([1, 16], F32, tag="pa")
                        nc.vector.tensor_copy(pb[:], pa[:])
                        pa = pb
            # ---- phase 0: h0 = relu(x @ Win) ----
            with (
                tc.tile_pool(name="p0s", bufs=3) as p0s,
                tc.tile_pool(name="p0p", bufs=2, space="PSUM") as p0p,
            ):
                for t0 in range(0, NT, 8):
                    xt = p0s.tile([FIN, 8, 128], F32, tag="xt")
                    nc.sync.dma_start(
                        xt[:], t_xT[:, t0 * 128 : (t0 + 8) * 128]
                    )
                    for j in range(8):
                        t = t0 + j
                        ph = p0p.tile([HID, 128], F32, tag="ph")
                        nc.tensor.matmul(
                            ph[:], Win[:], xt[:, j, :], start=True, stop=True
                        )
                        nc.scalar.activation(
                            h_resT[:, t * 128 : (t + 1) * 128], ph[:], AF.Relu
                        )

            for l in range(dbg_layers):
                # ---- P1: xl/al for all nodes -> gather table ----
                with (
                    tc.tile_pool(name="p1s", bufs=3) as p1s,
                    tc.tile_pool(name="p1p", bufs=2, space="PSUM") as p1p,
                ):
                    for t0 in range(0, NT, 4):
                        stage = p1s.tile([128, 4, DROW], BF16, tag="st")
                        pal4 = p1p.tile([128, 4, 2 * HEADS], F32, tag="al")
                        for j in range(4):
                            t = t0 + j
                            hT = h_resT[:, t * 128 : (t + 1) * 128]
                            pxl = p1p.tile([128, HEADS * HID], F32, tag="xl", bufs=3)
                            nc.tensor.matmul(
                                pxl[:], hT, Wl[:, l, :], start=True, stop=True
                            )
                            nc.tensor.matmul(
                                pal4[:, j, :], hT, WAl[:, l, :],
                                start=True, stop=True,
                            )
                            if t % 3 == 0:
                                nc.vector.tensor_copy(stage[:, j, 0:512], pxl[:])
                            else:
                                nc.scalar.copy(stage[:, j, 0:512], pxl[:])
                        nc.vector.tensor_copy(
                            stage[:, :, 512:520].bitcast(F32),
                            pal4[:, :, 0:HEADS],
                        )
                        nc.vector.tensor_copy(
                            al_dst_all[:, t0 : t0 + 4, :],
                            pal4[:, :, HEADS : 2 * HEADS],
                        )
                        nc.sync.dma_start(
                            table[t0 * 128 : (t0 + 4) * 128, 0:520].rearrange(
                                "(j p) c -> p j c", j=4
                            ),
                            stage[:, :, 0:520],
                        )
                    # own slice of al_dst (core-dependent via register offset)
                    nc.vector.tensor_copy(
                        al_dst_own[:, :, :],
                        al_dst_all[:, bass.ds(pid * TPC, TPC), :],
                    )

                if dbg_stop == "p1":
                    break
                # ---- P2: per own dst tile: gather + attention + scatter ----
                with (
                    tc.tile_pool(name="p2s", bufs=2) as p2s,
                    tc.tile_pool(name="p2p", bufs=2, space="PSUM") as p2p,
                ):
                    for tl in range(TPC):
                        X = p2s.tile([128, nblk, DROW], BF16, tag="X", bufs=3)
                        for g0 in range(0, nblk, 4):
                            g1 = min(g0 + 4, nblk)
                            nc.gpsimd.dma_gather(
                                X[:, g0:g1, :],
                                table[:],
                                gidx[
                                    :,
                                    tl * nblk * 8 + g0 * 8 : tl * nblk * 8 + g1 * 8,
                                ],
                                (g1 - g0) * 128,
                                (g1 - g0) * 128,
                                DROW,
                            )
                        pad = p2p.tile([128, nblk * HEADS], F32, tag="ad")
                        pout = p2p.tile([128, HEADS * HID], F32, tag="out")
                        pz = p2p.tile([128, HEADS], F32, tag="z", bufs=1)
                        Sb_l = []
                        for b in range(nblk):
                            dcol = dcols[:, tl * nblk + b : tl * nblk + b + 1]
                            drows = p2p.tile([128, 128], F32, tag="dr")
                            nc.tensor.transpose(
                                drows[:], dcol.to_broadcast([128, 128]), ident[:]
                            )
                            Sb = p2s.tile([128, 128], BF16, tag=f"Sb{b % 3}")
                            SbT = p2s.tile([128, 128], BF16, tag=f"SbT{b % 3}")
                            nc.vector.tensor_scalar(
                                Sb[:], iota_rowb[:], dcol, None, ALU.is_equal
                            )
                            nc.vector.tensor_scalar(
                                SbT[:], drows[:], iota_col[:], None, ALU.is_equal
                            )
                            nc.tensor.matmul(
                                pad[:, b * HEADS : (b + 1) * HEADS],
                                SbT[:],
                                al_dst_own[:, tl, :],
                                start=True,
                                stop=True,
                            )
                            Sb_l.append(Sb)
                        # attention weights for the whole tile
                        ew = p2s.tile([128, nblk * HEADS], F32, tag="ew")
                        nc.vector.tensor_add(
                            ew[:],
                            X[:, :, 512:520].bitcast(F32),
                            pad[:],
                        )
                        nc.vector.scalar_tensor_tensor(
                            ew[:], ew[:], NEG, ew[:], ALU.mult, ALU.max
                        )
                        nc.scalar.activation(ew[:], ew[:], AF.Exp)
                        ewr = p2s.tile([128, nblk * HEADS], BF16, tag="ewr")
                        nc.vector.tensor_copy(ewr[:], ew[:])
                        for b in range(nblk):
                            wX = p2s.tile([128, HEADS * HID], BF16, tag=f"wX{b % 3}")
                            for hh in range(HEADS):
                                xs = X[:, b, hh * HID : (hh + 1) * HID]
                                wcol = ew[:, b * HEADS + hh : b * HEADS + hh + 1]
                                if hh < 2:
                                    nc.vector.tensor_scalar(
                                        wX[:, hh * HID : (hh + 1) * HID],
                                        xs, wcol, None, ALU.mult,
                                    )
                                else:
                                    nc.scalar.activation(
                                        wX[:, hh * HID : (hh + 1) * HID],
                                        xs, AF.Copy, scale=wcol,
                                    )
                            nc.tensor.matmul(
                                pout[:],
                                Sb_l[b][:],
                                wX[:],
                                start=(b == 0),
                                stop=(b == nblk - 1),
                            )
                            nc.tensor.matmul(
                                pz[:],
                                Sb_l[b][:],
                                ewr[:, b * HEADS : (b + 1) * HEADS],
                                start=(b == 0),
                                stop=(b == nblk - 1),
                            )
                        # divide by z, mean over heads, bias, relu, residual
                        zc = p2s.tile([128, HEADS], F32, tag="zc")
                        nc.vector.tensor_scalar(zc[:], pz[:], 1e-30, None, ALU.max)
                        zr = p2s.tile([128, HEADS], F32, tag="zr")
                        nc.vector.reciprocal(zr[:], zc[:])
                        nc.vector.tensor_scalar(zr[:], zr[:], 0.25, None, ALU.mult)
                        acc = p2s.tile([128, HID], F32, tag="acc")
                        nc.vector.tensor_scalar(
                            acc[:], pout[:, 0:HID], zr[:, 0:1], None, ALU.mult
                        )
                        for hh in range(1, HEADS):
                            nc.vector.scalar_tensor_tensor(
                                acc[:],
                                pout[:, hh * HID : (hh + 1) * HID],
                                zr[:, hh : hh + 1],
                                acc[:],
                                ALU.mult,
                                ALU.add,
                            )
                        accT = p2p.tile([HID, 128], F32, tag="accT", bufs=1)
                        nc.tensor.transpose(accT[:], acc[:], ident[:])
                        hr = p2s.tile([HID, 128], F32R, tag="hr")
                        nc.scalar.activation(
                            hr[:], accT[:], AF.Relu, bias=bcol[:, l : l + 1]
                        )
                        hn = p2s.tile([HID, 128], F32R, tag="hn")
                        nc.vector.tensor_add(
                            hn[:],
                            hr[:],
                            h_resT[:, bass.ds((pid * TPC + tl) * 128, 128)],
                        )
                        nc.sync.dma_start(cc_in[tl], hn[:])

                if dbg_stop == "p2":
                    break
                if dbg_stop in ("nocc", "sim"):
                    continue
                # ---- P3: allgather h ----
                nc.gpsimd.collective_compute(
                    "AllGather",
                    ALU.bypass,
                    replica_groups=[list(range(NCORE))],
                    ins=[cc_in[:, :, :].opt()],
                    outs=[ag_outs[l][:, :, :].opt()],
                )
                nc.sync.dma_start(
                    h_resT[:, :].rearrange("c (t n) -> c t n", t=NT),
                    ag_outs[l][:, :, :].rearrange("t c n -> c t n"),
                )

            with tc.tile_pool(name="hd", bufs=1) as hdp:
                hd = hdp.tile([128, 8 * HID], F32)
                for i in range(8):
                    nc.vector.tensor_copy(
                        hd[:, i * HID : (i + 1) * HID],
                        h_resT[
                            :, i * TPC_DUMP * 128 : (i * TPC_DUMP + 1) * 128
                        ].bitcast(F32),
                    )
                nc.sync.dma_start(o_h[:], hd[:])
            if dbg_stop in ("p1", "p2", "p3", "sim"):
                return nc
            # ---- P4: graph mean pool + MLP ----
            with (
                tc.tile_pool(name="p4s", bufs=2) as p4s,
                tc.tile_pool(name="p4p", bufs=1, space="PSUM") as p4p,
            ):
                h_ownT = p4s.tile([128, TPC, 128], F32R)
                nc.sync.dma_start(
                    h_ownT[:, :, :], cc_in[:, :, :].rearrange("t c n -> c t n")
                )
                ppool = p4p.tile([32, HID], F32, tag="pool")
                for tl in range(TPC):
                    hnm_ps = p4p.tile([128, HID], F32R, tag="hnm")
                    nc.tensor.transpose(
                        hnm_ps[:], h_ownT[:, tl, :], identr[:]
                    )
                    hnm = p4s.tile([128, HID], F32R, tag="hnm")
                    nc.vector.tensor_copy(hnm[:], hnm_ps[:])
                    nc.tensor.matmul(
                        ppool[:],
                        btile[:, tl * 32 : (tl + 1) * 32],
                        hnm[:],
                        start=(tl == 0),
                        stop=(tl == TPC - 1),
                    )
                pool_sb = p4s.tile([32, HID], F32)
                nc.vector.tensor_copy(pool_sb[:], ppool[:])
                nc.sync.dma_start(ar_in[:], pool_sb[:])
                nc.gpsimd.collective_compute(
                    "AllReduce",
                    ALU.add,
                    replica_groups=[list(range(NCORE))],
                    ins=[ar_in[:].opt()],
                    outs=[ar_out[:].opt()],
                )
                g_sb = p4s.tile([G, HID], F32)
                nc.sync.dma_start(g_sb[:], ar_out[:])

                def t_r(src_ap, pdim, fdim, tag):
                    """transpose + round to f32r: [pdim,fdim] -> [fdim,pdim]"""
                    ps = p4p.tile([fdim, pdim], F32, tag=tag + "p")
                    nc.tensor.transpose(ps[:], src_ap, ident[:pdim, :pdim])
                    sb = p4s.tile([fdim, pdim], F32R, tag=tag)
                    nc.vector.tensor_copy(sb[:], ps[:])
                    return sb

                W1 = p4s.tile([HID, 64], F32R)
                W2 = p4s.tile([64, 64], F32R)
                W3 = p4s.tile([64, 32], F32R)
                b1 = p4s.tile([32, 64], F32)
                b2 = p4s.tile([32, 64], F32)
                b3 = p4s.tile([32, 32], F32)
                nc.sync.dma_start(W1[:], t_W1[:])
                nc.sync.dma_start(W2[:], t_W2[:])
                nc.sync.dma_start(W3[:], t_W3[:])
                nc.sync.dma_start(b1[:], t_b1[:])
                nc.sync.dma_start(b2[:], t_b2[:])
                nc.sync.dma_start(b3[:], t_b3[:])

                gT = t_r(g_sb[:], G, HID, "gT")              # [128, 32]
                pm1 = p4p.tile([G, 64], F32, tag="m1")
                nc.tensor.matmul(pm1[:], gT[:], W1[:], start=True, stop=True)
                o1 = p4s.tile([G, 64], F32, tag="o1")
                nc.vector.tensor_add(o1[:], pm1[:], b1[:])
                nc.scalar.activation(o1[:], o1[:], AF.Relu)

                o1T = t_r(o1[:], G, 64, "o1T")               # [64, 32]
                pm2 = p4p.tile([G, 64], F32, tag="m2")
                nc.tensor.matmul(pm2[:], o1T[:], W2[:], start=True, stop=True)
                o2 = p4s.tile([G, 64], F32, tag="o2")
                nc.vector.tensor_add(o2[:], pm2[:], b2[:])
                nc.scalar.activation(o2[:], o2[:], AF.Relu)

                o2T = t_r(o2[:], G, 64, "o2T")               # [64, 32]
                pm3 = p4p.tile([G, 32], F32, tag="m3")
                nc.tensor.matmul(pm3[:], o2T[:], W3[:], start=True, stop=True)
                o3 = p4s.tile([G, 32], F32, tag="o3")
                nc.vector.tensor_add(o3[:], pm3[:], b3[:])
                nc.sync.dma_start(o_out[:], o3[:])
    return nc


_CACHE = {}
_LAST_NBLK = 11


def _get_program(nblk):
    if nblk not in _CACHE:
        nc = _build(nblk)
        _split_waits(nc)
        nc.compile()
        _CACHE[nblk] = nc
    return _CACHE[nblk]


def kernel(**inputs):
    import os

    inp = {k: np.asarray(v) for k, v in inputs.items()}
    prep = _preprocess(
        inp["x"].astype(np.float32),
        inp["edge_index"].astype(np.int64),
        inp["batch"].astype(np.int64),
        inp["gat_W"].astype(np.float32),
        inp["att_src"].astype(np.float32),
        inp["att_dst"].astype(np.float32),
    )
    nblk = prep["nblk"]
    global _LAST_NBLK
    _LAST_NBLK = nblk
    nc = _get_program(nblk)

    iota_row = np.tile(np.arange(128, dtype=np.float32)[None, :], (128, 1))
    iota_col = np.arange(128, dtype=np.float32)[:, None].copy()
    ident = np.eye(128, dtype=np.float32)
    bcol = inp["gat_b"].astype(np.float32)[:, :, None]  # [L,HID,1]
    b1t = np.tile(inp["b1"].astype(np.float32)[None, :], (32, 1))
    b2t = np.tile(inp["b2"].astype(np.float32)[None, :], (32, 1))
    b3t = np.tile(inp["b3"].astype(np.float32)[None, :], (32, 1))

    bt = mybir.dt.np(mybir.dt.bfloat16)
    shared = dict(
        xT=prep["xT"],
        iota_row=iota_row,
        iota_rowb=iota_row.astype(bt),
        iota_col=iota_col,
        ident=ident,
        identr=ident,
        Win=inp["W_in"].astype(np.float32),
        Wl=inp["gat_W"].astype(np.float32),
        WAl=prep["WA"],
        bcol=bcol,
        W1=inp["W1"].astype(np.float32),
        W2=inp["W2"].astype(np.float32),
        W3=inp["W3"].astype(np.float32),
        b1t=b1t,
        b2t=b2t,
        b3t=b3t,
    )
    in_maps = []
    for c in range(NCORE):
        m = dict(shared)
        m["gidx"] = prep["gidx"][c]
        m["dcols"] = prep["dcols"][c]
        m["dcolsb"] = prep["dcols"][c].astype(bt)
        m["btile"] = prep["btile"][c]
        in_maps.append(m)

    trace = bool(int(os.environ.get("KERNEL_TRACE", "0")))
    last_exc = None
    for attempt in range(3):
        try:
            res = run_bass_kernel_spmd(
                nc, in_maps, core_ids=list(range(NCORE)), trace=trace
            )
            break
        except Exception as exc:  # transient device-unrecoverable after crashes
            last_exc = exc
            import time as _time

            _time.sleep(15)
    else:
        raise last_exc
    if trace and res.exec_time_ns is not None:
        print(f"HW exec time: {res.exec_time_ns} ns")
        kernel.last_exec_time_ns = res.exec_time_ns
        kernel.last_trace = res.instructions_and_trace
    return np.asarray(res.results[0]["out"], dtype=np.float32)


# revision 35
# speedup vs baseline: 1.5417x; 1.0790x over previous
"""GAT message-passing GNN on 8 Trainium2 NeuronCores (Bass/Tile).

Strategy: nodes are permuted (degree-balanced, round-robin over 160 tiles of
125 nodes) and partitioned across 8 cores (20 dst tiles each). Each layer:
every core redundantly computes xl = h @ W (and attention logits al = h @ WA)
for all nodes into a DRAM gather table; each core then processes its own dst
tiles: one dma_gather fetches xl[src] (+al_src) for all incident edges,
per-128-edge-block one-hot dst matrices are built with is_equal compares, and
the segment softmax + weighted scatter-add run as f32r matmuls accumulating in
PSUM (w = exp(leakyrelu(as+ad)) per edge; out = S^T @ (w*X); z = S^T @ w;
divide by z once per dst node). Updated node features are AllGathered each
layer. Final graph mean-pool is a one-hot matmul + AllReduce, then the MLP.
"""
import numpy as np

import concourse.bass as bass
import concourse.bacc as bacc
import concourse.mybir as mybir
import concourse.tile as tile
from concourse.bass_utils import run_bass_kernel_spmd

F32 = mybir.dt.float32
F32R = mybir.dt.float32r
BF16 = mybir.dt.bfloat16
I16 = mybir.dt.int16
AF = mybir.ActivationFunctionType
ALU = mybir.AluOpType

N, E, FIN, HID, HEADS, L, G = 20000, 200000, 20, 128, 4, 4, 32
NEG = 0.2
NCORE = 8
NT = 160            # global dst tiles
TPC = NT // NCORE   # 20 tiles per core
TILE_N = N // NT    # 125 real nodes per tile
PN = NT * 128       # padded node id space; PN = zero row
DROW = 640          # bf16 table row: 512 xl + 8 (4 f32 al_src) + pad (1280B, %256==0)
PADDST = 999.0
TPC_DUMP = 20  # dump tiles 0,20,40,... (first tile of each core)

_ZERO_WAIT_OPCODES = (
    "InstDMAGatherAnt",
    "InstDMAScatterAddAnt",
    "InstPartitionBroadcast",
    "InstPartitionAllReduce",
    "InstAPGather",
    "InstIndirectCopy",
    "InstSparseGather",
    "InstGatherTranspose",
)
_spill_counter = [0]


def _split_waits(nc, default_limit=1):
    """Spill excess semaphore waits onto preceding same-engine EventSemaphore
    instructions (walrus wait-slot limits: 0 for extended DMA ops, ~1+ else)."""
    for f in nc.m.functions:
        for bb in f.blocks:
            out = []
            changed = False
            for ins in bb.instructions:
                si = ins.sync_info
                waits = list(si.on_wait) if si is not None and si.on_wait else []
                tname = type(ins).__name__
                limit = default_limit
                if tname in _ZERO_WAIT_OPCODES:
                    limit = 0
                elif ins.engine == mybir.EngineType.Pool and tname in (
                    "InstDrain",
                    "InstNoOp",
                ):
                    limit = 0
                if len(waits) > limit:
                    changed = True
                    keep = waits[:limit] if limit else []
                    spill = waits[limit:] if limit else waits
                    while spill:
                        chunk, spill = spill[:1], spill[1:]
                        _spill_counter[0] += 1
                        nop = mybir.InstEventSemaphore(
                            name=f"waitspill-{_spill_counter[0]}"
                        )
                        nop.engine = ins.engine
                        nop.sync_info = mybir.SyncInfo(on_wait=chunk, on_update=[])
                        nc.register_instruction(nop, overwrite=True)
                        out.append(nop)
                    ins.sync_info = mybir.SyncInfo(
                        on_wait=keep, on_update=list(si.on_update) if si else []
                    )
                out.append(ins)
            if changed:
                bb.instructions[:] = out


def _preprocess(x, edge_index, batch, gat_W, att_src, att_dst):
    """Degree-balanced node permutation + per-core edge/tile data."""
    src = np.concatenate([edge_index[0], np.arange(N, dtype=np.int64)])
    dst = np.concatenate([edge_index[1], np.arange(N, dtype=np.int64)])
    indeg = np.bincount(dst, minlength=N)
    order = np.argsort(-indeg, kind="stable")
    new_id = np.empty(N, dtype=np.int64)
    ranks = np.arange(N)
    new_id[order] = (ranks % NT) * 128 + (ranks // NT)

    nsrc = new_id[src]
    ndst = new_id[dst]
    tile_e = ndst >> 7
    dloc = ndst & 127
    eorder = np.argsort(tile_e, kind="stable")
    tile_sorted = tile_e[eorder]
    nsrc_sorted = nsrc[eorder]
    dloc_sorted = dloc[eorder]
    starts = np.searchsorted(tile_sorted, np.arange(NT + 1))
    cnts = np.diff(starts)
    nblk = int(np.ceil(cnts.max() / 128))
    ET = nblk * 128

    gsrc = np.full((NT, ET), PN, dtype=np.int64)
    gdst = np.full((NT, ET), int(PADDST), dtype=np.int64)
    for t in range(NT):
        s, c = starts[t], cnts[t]
        gsrc[t, :c] = nsrc_sorted[s : s + c]
        gdst[t, :c] = dloc_sorted[s : s + c]

    # per-core arrays
    gidx = np.zeros((NCORE, 128, TPC * nblk * 8), dtype=np.int16)
    dcols = np.zeros((NCORE, 128, TPC * nblk), dtype=np.float32)
    for c in range(NCORE):
        for tl in range(TPC):
            t = c * TPC + tl
            wrap = gsrc[t].astype(np.int16).reshape(ET // 16, 16).T  # [16, ET/16]
            gidx[c, :, tl * nblk * 8 : (tl + 1) * nblk * 8] = np.tile(wrap, (8, 1))
            dcols[c, :, tl * nblk : (tl + 1) * nblk] = (
                gdst[t].reshape(nblk, 128).T.astype(np.float32)
            )

    # pooling matrix with 1/cnt folded in
    cnt = np.bincount(batch, minlength=G).astype(np.float32)
    cnt = np.maximum(cnt, 1.0)
    btile = np.zeros((NCORE, 128, TPC * 32), dtype=np.float32)
    inv = np.zeros(PN, dtype=np.int64)
    inv[new_id] = np.arange(N)  # new -> old (only valid slots)
    valid = np.zeros(PN, dtype=bool)
    valid[new_id] = True
    for c in range(NCORE):
        for tl in range(TPC):
            t = c * TPC + tl
            for p in range(TILE_N):
                nid = t * 128 + p
                if valid[nid]:
                    n_old = inv[nid]
                    g = batch[n_old]
                    btile[c, p, tl * 32 + g] = 1.0 / cnt[g]

    # permuted transposed input features
    xT = np.zeros((FIN, PN), dtype=np.float32)
    xT[:, new_id] = x.T

    # folded attention projections WA_l = W_l @ [A_src | A_dst]
    WA = np.zeros((L, HID, 2 * HEADS), dtype=np.float32)
    for l in range(L):
        A = np.zeros((HID * HEADS, 2 * HEADS), dtype=np.float64)
        for h in range(HEADS):
            A[h * HID : (h + 1) * HID, h] = att_src[l][h]
            A[h * HID : (h + 1) * HID, HEADS + h] = att_dst[l][h]
        WA[l] = (gat_W[l].astype(np.float64) @ A).astype(np.float32)

    return dict(gidx=gidx, dcols=dcols, btile=btile, xT=xT, WA=WA, nblk=nblk)


def _build(nblk, dbg_stop=None, dbg_layers=L, dbg_pad=0):
    ET = nblk * 128
    nc = bacc.Bacc("TRN2", target_bir_lowering=False, debug=False, num_devices=NCORE)

    t_xT = nc.dram_tensor("xT", [FIN, PN], F32, kind="ExternalInput")
    t_gidx = nc.dram_tensor("gidx", [128, TPC * nblk * 8], I16, kind="ExternalInput")
    t_dcols = nc.dram_tensor("dcols", [128, TPC * nblk], F32, kind="ExternalInput")
    t_dcolsb = nc.dram_tensor("dcolsb", [128, TPC * nblk], BF16, kind="ExternalInput")
    t_iota_rowb = nc.dram_tensor("iota_rowb", [128, 128], BF16, kind="ExternalInput")
    t_btile = nc.dram_tensor("btile", [128, TPC * 32], F32R, kind="ExternalInput")
    t_iota_row = nc.dram_tensor("iota_row", [128, 128], F32, kind="ExternalInput")
    t_iota_col = nc.dram_tensor("iota_col", [128, 1], F32, kind="ExternalInput")
    t_ident = nc.dram_tensor("ident", [128, 128], F32, kind="ExternalInput")
    t_identr = nc.dram_tensor("identr", [128, 128], F32R, kind="ExternalInput")
    t_Win = nc.dram_tensor("Win", [FIN, HID], F32, kind="ExternalInput")
    t_Wl = nc.dram_tensor("Wl", [L, HID, HEADS * HID], F32R, kind="ExternalInput")
    t_WAl = nc.dram_tensor("WAl", [L, HID, 2 * HEADS], F32R, kind="ExternalInput")
    t_bcol = nc.dram_tensor("bcol", [L, HID, 1], F32, kind="ExternalInput")
    t_W1 = nc.dram_tensor("W1", [HID, 64], F32R, kind="ExternalInput")
    t_W2 = nc.dram_tensor("W2", [64, 64], F32R, kind="ExternalInput")
    t_W3 = nc.dram_tensor("W3", [64, 32], F32R, kind="ExternalInput")
    t_b1 = nc.dram_tensor("b1t", [32, 64], F32, kind="ExternalInput")
    t_b2 = nc.dram_tensor("b2t", [32, 64], F32, kind="ExternalInput")
    t_b3 = nc.dram_tensor("b3t", [32, 32], F32, kind="ExternalInput")
    o_out = nc.dram_tensor("out", [G, 32], F32, kind="ExternalOutput")
    o_h = nc.dram_tensor("hdump", [128, 8 * HID], F32, kind="ExternalOutput")

    table = nc.dram_tensor("table", [PN + 1, DROW], BF16)

    with tile.TileContext(nc) as tc:
        with (
            tc.tile_pool(name="const", bufs=1) as cpool,
            tc.tile_pool(name="persist", bufs=1) as hpool,
            tc.tile_pool(name="dram", bufs=1, space="DRAM") as dpool,
        ):
            # ---- constants to SBUF ----
            iota_row = cpool.tile([128, 128], F32)
            iota_col = cpool.tile([128, 1], F32)
            ident = cpool.tile([128, 128], F32)
            identr = cpool.tile([128, 128], F32R)
            Win = cpool.tile([FIN, HID], F32)
            Wl = cpool.tile([128, L, HEADS * HID], F32R)
            WAl = cpool.tile([128, L, 2 * HEADS], F32R)
            bcol = cpool.tile([HID, L], F32)
            gidx = cpool.tile([128, TPC * nblk * 8], I16)
            dcols = cpool.tile([128, TPC * nblk], F32)
            dcolsb = cpool.tile([128, TPC * nblk], BF16)
            iota_rowb = cpool.tile([128, 128], BF16)
            btile = cpool.tile([128, TPC * 32], F32R)
            nc.sync.dma_start(iota_row[:], t_iota_row[:])
            nc.sync.dma_start(iota_col[:], t_iota_col[:])
            nc.sync.dma_start(ident[:], t_ident[:])
            nc.sync.dma_start(identr[:], t_identr[:])
            nc.sync.dma_start(Win[:], t_Win[:])
            for l in range(L):
                nc.sync.dma_start(Wl[:, l, :], t_Wl[l])
                nc.sync.dma_start(WAl[:, l, :], t_WAl[l])
                nc.sync.dma_start(bcol[:, l : l + 1], t_bcol[l])
            nc.gpsimd.dma_start(gidx[:], t_gidx[:])
            nc.sync.dma_start(dcols[:], t_dcols[:])
            nc.sync.dma_start(dcolsb[:], t_dcolsb[:])
            nc.sync.dma_start(iota_rowb[:], t_iota_rowb[:])
            nc.sync.dma_start(btile[:], t_btile[:])

            # persistent node features [p, tile, c]
            h_resT = hpool.tile([128, NT * 128], F32R)
            al_dst_all = hpool.tile([128, NT, HEADS], BF16)
            al_dst_own = hpool.tile([128, TPC, HEADS], BF16)

            # zero row of the gather table
            with tc.tile_pool(name="zr", bufs=1) as zpool:
                zrow = zpool.tile([1, DROW], BF16)
                nc.vector.memset(zrow[:], 0.0)
                nc.sync.dma_start(table[PN : PN + 1, :], zrow[:])

            # collective buffers
            cc_in = dpool.tile([TPC, 128, HID], F32R)
            ag_outs = [
                dpool.tile([NT, 128, HID], F32R, addr_space="Shared", name=f"ag{i}")
                for i in range(L)
            ]
            ar_in = dpool.tile([G, HID], F32)
            ar_out = dpool.tile([G, HID], F32, addr_space="Shared")

            pid = nc.vector.partition_id()

            if dbg_pad:
                with tc.tile_pool(name="padp", bufs=2) as padp:
                    pa = padp.tile([1, 16], F32, tag="pa")
                    nc.vector.memset(pa[:], 0.0)
                    for _ in range(dbg_pad):
                        pb = padp.tile([1, 16], F32, tag="pa")
                        nc.vector.tensor_copy(pb[:], pa[:])
                        pa = pb
            # ---- phase 0: h0 = relu(x @ Win) ----
            with (
                tc.tile_pool(name="p0s", bufs=3) as p0s,
                tc.tile_pool(name="p0p", bufs=2, space="PSUM") as p0p,
            ):
                for t0 in range(0, NT, 8):
                    xt = p0s.tile([FIN, 8, 128], F32, tag="xt")
                    nc.sync.dma_start(
                        xt[:], t_xT[:, t0 * 128 : (t0 + 8) * 128]
                    )
                    for j in range(8):
                        t = t0 + j
                        ph = p0p.tile([HID, 128], F32, tag="ph")
                        nc.tensor.matmul(
                            ph[:], Win[:], xt[:, j, :], start=True, stop=True
                        )
                        nc.scalar.activation(
                            h_resT[:, t * 128 : (t + 1) * 128], ph[:], AF.Relu
                        )

            for l in range(dbg_layers):
                # ---- P1: xl/al for all nodes -> gather table ----
                with (
                    tc.tile_pool(name="p1s", bufs=3) as p1s,
                    tc.tile_pool(name="p1p", bufs=2, space="PSUM") as p1p,
                ):
                    for t0 in range(0, NT, 4):
                        stage = p1s.tile([128, 4, DROW], BF16, tag="st", bufs=4)
                        pal4 = p1p.tile([128, 4, 2 * HEADS], F32, tag="al")
                        for j in range(4):
                            t = t0 + j
                            hT = h_resT[:, t * 128 : (t + 1) * 128]
                            pxl = p1p.tile([128, HEADS * HID], F32, tag="xl", bufs=3)
                            nc.tensor.matmul(
                                pxl[:], hT, Wl[:, l, :], start=True, stop=True
                            )
                            nc.tensor.matmul(
                                pal4[:, j, :], hT, WAl[:, l, :],
                                start=True, stop=True,
                            )
                            if t % 2 == 0:
                                nc.vector.tensor_copy(stage[:, j, 0:512], pxl[:])
                            else:
                                nc.scalar.copy(stage[:, j, 0:512], pxl[:])
                        nc.vector.tensor_copy(
                            stage[:, :, 512:520].bitcast(F32),
                            pal4[:, :, 0:HEADS],
                        )
                        nc.vector.tensor_copy(
                            al_dst_all[:, t0 : t0 + 4, :],
                            pal4[:, :, HEADS : 2 * HEADS],
                        )
                        nc.sync.dma_start(
                            table[t0 * 128 : (t0 + 4) * 128, 0:520].rearrange(
                                "(j p) c -> p j c", j=4
                            ),
                            stage[:, :, 0:520],
                        )
                    # own slice of al_dst (core-dependent via register offset)
                    nc.vector.tensor_copy(
                        al_dst_own[:, :, :],
                        al_dst_all[:, bass.ds(pid * TPC, TPC), :],
                    )

                if dbg_stop == "p1":
                    break
                # ---- P2: per own dst tile: gather + attention + scatter ----
                with (
                    tc.tile_pool(name="p2s", bufs=2) as p2s,
                    tc.tile_pool(name="p2p", bufs=2, space="PSUM") as p2p,
                ):
                    for tl in range(TPC):
                        X = p2s.tile([128, nblk, DROW], BF16, tag="X", bufs=4)
                        for g0 in range(0, nblk, 4):
                            g1 = min(g0 + 4, nblk)
                            nc.gpsimd.dma_gather(
                                X[:, g0:g1, :],
                                table[:],
                                gidx[
                                    :,
                                    tl * nblk * 8 + g0 * 8 : tl * nblk * 8 + g1 * 8,
                                ],
                                (g1 - g0) * 128,
                                (g1 - g0) * 128,
                                DROW,
                            )
                        pad = p2p.tile([128, nblk * HEADS], F32, tag="ad")
                        pout = p2p.tile([128, HEADS * HID], F32, tag="out")
                        pz = p2p.tile([128, HEADS], F32, tag="z", bufs=1)
                        Sb_l = []
                        for b in range(nblk):
                            dcol = dcols[:, tl * nblk + b : tl * nblk + b + 1]
                            drows = p2p.tile([128, 128], F32, tag="dr")
                            nc.tensor.transpose(
                                drows[:], dcol.to_broadcast([128, 128]), ident[:]
                            )
                            Sb = p2s.tile([128, 128], BF16, tag=f"Sb{b % 3}")
                            SbT = p2s.tile([128, 128], BF16, tag=f"SbT{b % 3}")
                            nc.vector.tensor_scalar(
                                Sb[:], iota_rowb[:], dcol, None, ALU.is_equal
                            )
                            nc.vector.tensor_scalar(
                                SbT[:], drows[:], iota_col[:], None, ALU.is_equal
                            )
                            nc.tensor.matmul(
                                pad[:, b * HEADS : (b + 1) * HEADS],
                                SbT[:],
                                al_dst_own[:, tl, :],
                                start=True,
                                stop=True,
                            )
                            Sb_l.append(Sb)
                        # attention weights for the whole tile
                        ew = p2s.tile([128, nblk * HEADS], F32, tag="ew")
                        nc.vector.tensor_add(
                            ew[:],
                            X[:, :, 512:520].bitcast(F32),
                            pad[:],
                        )
                        nc.vector.scalar_tensor_tensor(
                            ew[:], ew[:], NEG, ew[:], ALU.mult, ALU.max
                        )
                        nc.scalar.activation(ew[:], ew[:], AF.Exp)
                        ewr = p2s.tile([128, nblk * HEADS], BF16, tag="ewr")
                        nc.vector.tensor_copy(ewr[:], ew[:])
                        for b in range(nblk):
                            wX = p2s.tile([128, HEADS * HID], BF16, tag=f"wX{b % 3}")
                            for hh in range(HEADS):
                                xs = X[:, b, hh * HID : (hh + 1) * HID]
                                wcol = ew[:, b * HEADS + hh : b * HEADS + hh + 1]
                                if hh < 2:
                                    nc.vector.tensor_scalar(
                                        wX[:, hh * HID : (hh + 1) * HID],
                                        xs, wcol, None, ALU.mult,
                                    )
                                else:
                                    nc.scalar.activation(
                                        wX[:, hh * HID : (hh + 1) * HID],
                                        xs, AF.Copy, scale=wcol,
                                    )
                            nc.tensor.matmul(
                                pout[:],
                                Sb_l[b][:],
                                wX[:],
                                start=(b == 0),
                                stop=(b == nblk - 1),
                            )
                            nc.tensor.matmul(
                                pz[:],
                                Sb_l[b][:],
                                ewr[:, b * HEADS : (b + 1) * HEADS],
                                start=(b == 0),
                                stop=(b == nblk - 1),
                            )
                        # divide by z, mean over heads, bias, relu, residual
                        zc = p2s.tile([128, HEADS], F32, tag="zc")
                        nc.vector.tensor_scalar(zc[:], pz[:], 1e-30, None, ALU.max)
                        zr = p2s.tile([128, HEADS], F32, tag="zr")
                        nc.vector.reciprocal(zr[:], zc[:])
                        nc.vector.tensor_scalar(zr[:], zr[:], 0.25, None, ALU.mult)
                        acc = p2s.tile([128, HID], F32, tag="acc")
                        nc.vector.tensor_scalar(
                            acc[:], pout[:, 0:HID], zr[:, 0:1], None, ALU.mult
                        )
                        for hh in range(1, HEADS):
                            nc.vector.scalar_tensor_tensor(
                                acc[:],
                                pout[:, hh * HID : (hh + 1) * HID],
                                zr[:, hh : hh + 1],
                                acc[:],
                                ALU.mult,
                                ALU.add,
                            )
                        accT = p2p.tile([HID, 128], F32, tag="accT", bufs=1)
                        nc.tensor.transpose(accT[:], acc[:], ident[:])
                        hr = p2s.tile([HID, 128], F32R, tag="hr")
                        nc.scalar.activation(
                            hr[:], accT[:], AF.Relu, bias=bcol[:, l : l + 1]
                        )
                        hn = p2s.tile([HID, 128], F32R, tag="hn")
                        nc.vector.tensor_add(
                            hn[:],
                            hr[:],
                            h_resT[:, bass.ds((pid * TPC + tl) * 128, 128)],
                        )
                        nc.sync.dma_start(cc_in[tl], hn[:])

                if dbg_stop == "p2":
                    break
                if dbg_stop in ("nocc", "sim"):
                    continue
                # ---- P3: allgather h ----
                nc.gpsimd.collective_compute(
                    "AllGather",
                    ALU.bypass,
                    replica_groups=[list(range(NCORE))],
                    ins=[cc_in[:, :, :].opt()],
                    outs=[ag_outs[l][:, :, :].opt()],
                )
                nc.sync.dma_start(
                    h_resT[:, :].rearrange("c (t n) -> c t n", t=NT),
                    ag_outs[l][:, :, :].rearrange("t c n -> c t n"),
                )

            with tc.tile_pool(name="hd", bufs=1) as hdp:
                hd = hdp.tile([128, 8 * HID], F32)
                nc.vector.memset(hd[:], 0.0)
                nc.sync.dma_start(o_h[:], hd[:])
            if dbg_stop in ("p1", "p2", "p3", "sim"):
                return nc
            # ---- P4: graph mean pool + MLP ----
            with (
                tc.tile_pool(name="p4s", bufs=2) as p4s,
                tc.tile_pool(name="p4p", bufs=1, space="PSUM") as p4p,
            ):
                h_ownT = p4s.tile([128, TPC, 128], F32R)
                nc.sync.dma_start(
                    h_ownT[:, :, :], cc_in[:, :, :].rearrange("t c n -> c t n")
                )
                ppool = p4p.tile([32, HID], F32, tag="pool")
                for tl in range(TPC):
                    hnm_ps = p4p.tile([128, HID], F32R, tag="hnm")
                    nc.tensor.transpose(
                        hnm_ps[:], h_ownT[:, tl, :], identr[:]
                    )
                    hnm = p4s.tile([128, HID], F32R, tag="hnm")
                    nc.vector.tensor_copy(hnm[:], hnm_ps[:])
                    nc.tensor.matmul(
                        ppool[:],
                        btile[:, tl * 32 : (tl + 1) * 32],
                        hnm[:],
                        start=(tl == 0),
                        stop=(tl == TPC - 1),
                    )
                pool_sb = p4s.tile([32, HID], F32)
                nc.vector.tensor_copy(pool_sb[:], ppool[:])
                nc.sync.dma_start(ar_in[:], pool_sb[:])
                nc.gpsimd.collective_compute(
                    "AllReduce",
                    ALU.add,
                    replica_groups=[list(range(NCORE))],
                    ins=[ar_in[:].opt()],
                    outs=[ar_out[:].opt()],
                )
                g_sb = p4s.tile([G, HID], F32)
                nc.sync.dma_start(g_sb[:], ar_out[:])

                def t_r(src_ap, pdim, fdim, tag):
                    """transpose + round to f32r: [pdim,fdim] -> [fdim,pdim]"""
                    ps = p4p.tile([fdim, pdim], F32, tag=tag + "p")
                    nc.tensor.transpose(ps[:], src_ap, ident[:pdim, :pdim])
                    sb = p4s.tile([fdim, pdim], F32R, tag=tag)
                    nc.vector.tensor_copy(sb[:], ps[:])
                    return sb

                W1 = p4s.tile([HID, 64], F32R)
                W2 = p4s.tile([64, 64], F32R)
                W3 = p4s.tile([64, 32], F32R)
                b1 = p4s.tile([32, 64], F32)
                b2 = p4s.tile([32, 64], F32)
                b3 = p4s.tile([32, 32], F32)
                nc.sync.dma_start(W1[:], t_W1[:])
                nc.sync.dma_start(W2[:], t_W2[:])
                nc.sync.dma_start(W3[:], t_W3[:])
                nc.sync.dma_start(b1[:], t_b1[:])
                nc.sync.dma_start(b2[:], t_b2[:])
                nc.sync.dma_start(b3[:], t_b3[:])

                gT = t_r(g_sb[:], G, HID, "gT")              # [128, 32]
                pm1 = p4p.tile([G, 64], F32, tag="m1")
                nc.tensor.matmul(pm1[:], gT[:], W1[:], start=True, stop=True)
                o1 = p4s.tile([G, 64], F32, tag="o1")
                nc.vector.tensor_add(o1[:], pm1[:], b1[:])
                nc.scalar.activation(o1[:], o1[:], AF.Relu)

                o1T = t_r(o1[:], G, 64, "o1T")               # [64, 32]
                pm2 = p4p.tile([G, 64], F32, tag="m2")
                nc.tensor.matmul(pm2[:], o1T[:], W2[:], start=True, stop=True)
                o2 = p4s.tile([G, 64], F32, tag="o2")
                nc.vector.tensor_add(o2[:], pm2[:], b2[:])
                nc.scalar.activation(o2[:], o2[:], AF.Relu)

                o2T = t_r(o2[:], G, 64, "o2T")               # [64, 32]
                pm3 = p4p.tile([G, 32], F32, tag="m3")
                nc.tensor.matmul(pm3[:], o2T[:], W3[:], start=True, stop=True)
                o3 = p4s.tile([G, 32], F32, tag="o3")
                nc.vector.tensor_add(o3[:], pm3[:], b3[:])
                nc.sync.dma_start(o_out[:], o3[:])
    return nc


_CACHE = {}
_LAST_NBLK = 11


def _get_program(nblk):
    if nblk not in _CACHE:
        nc = _build(nblk)
        _split_waits(nc)
        nc.compile()
        _CACHE[nblk] = nc
    return _CACHE[nblk]


def kernel(**inputs):
    import os

    inp = {k: np.asarray(v) for k, v in inputs.items()}
    prep = _preprocess(
        inp["x"].astype(np.float32),
        inp["edge_index"].astype(np.int64),
        inp["batch"].astype(np.int64),
        inp["gat_W"].astype(np.float32),
        inp["att_src"].astype(np.float32),
        inp["att_dst"].astype(np.float32),
    )
    nblk = prep["nblk"]
    global _LAST_NBLK
    _LAST_NBLK = nblk
    nc = _get_program(nblk)

    iota_row = np.tile(np.arange(128, dtype=np.float32)[None, :], (128, 1))
    iota_col = np.arange(128, dtype=np.float32)[:, None].copy()
    ident = np.eye(128, dtype=np.float32)
    bcol = inp["gat_b"].astype(np.float32)[:, :, None]  # [L,HID,1]
    b1t = np.tile(inp["b1"].astype(np.float32)[None, :], (32, 1))
    b2t = np.tile(inp["b2"].astype(np.float32)[None, :], (32, 1))
    b3t = np.tile(inp["b3"].astype(np.float32)[None, :], (32, 1))

    bt = mybir.dt.np(mybir.dt.bfloat16)
    shared = dict(
        xT=prep["xT"],
        iota_row=iota_row,
        iota_rowb=iota_row.astype(bt),
        iota_col=iota_col,
        ident=ident,
        identr=ident,
        Win=inp["W_in"].astype(np.float32),
        Wl=inp["gat_W"].astype(np.float32),
        WAl=prep["WA"],
        bcol=bcol,
        W1=inp["W1"].astype(np.float32),
        W2=inp["W2"].astype(np.float32),
        W3=inp["W3"].astype(np.float32),
        b1t=b1t,
        b2t=b2t,
        b3t=b3t,
    )
    in_maps = []
    for c in range(NCORE):
        m = dict(shared)
        m["gidx"] = prep["gidx"][c]
        m["dcols"] = prep["dcols"][c]
        m["dcolsb"] = prep["dcols"][c].astype(bt)
        m["btile"] = prep["btile"][c]
        in_maps.append(m)

    trace = bool(int(os.environ.get("KERNEL_TRACE", "0")))
    last_exc = None
    for attempt in range(3):
        try:
            res = run_bass_kernel_spmd(
                nc, in_maps, core_ids=list(range(NCORE)), trace=trace
            )
            break
        except Exception as exc:  # transient device-unrecoverable after crashes
            last_exc = exc
            import time as _time

            _time.sleep(15)
    else:
        raise last_exc
    if trace and res.exec_time_ns is not None:
        print(f"HW exec time: {res.exec_time_ns} ns")
        kernel.last_exec_time_ns = res.exec_time_ns
        kernel.last_trace = res.instructions_and_trace
    return np.asarray(res.results[0]["out"], dtype=np.float32)
